# revision 8
# baseline (speedup 1.0000x reference)
"""Trainium2 Bass kernel for nn_CosSim_Loss.

Computes mean of per-batch cosine-similarity Gram matrices of
pred [32, 8, 512, 512] -> scalar.

Strategy: shard the contraction dim L = 512*512 = 262144 across the 8
cores (each core gets L/8 = 32768 contiguous elements of every row).
Each core computes the partial Gram sums D[m, n] = sum_l x[m, l] x[n, l]
for the two 128-row groups (rows = 32 batches x 8 maps = 256) with
TensorE matmuls (contraction on partitions, fp32->bf16 cast during the
DMA load), accumulating in PSUM over 256 k-chunks. The host sums the
8 per-core partial Grams, extracts the per-batch 8x8 diagonal blocks,
normalizes by the row norms (taken from the Gram diagonal) and takes
the mean, with the diagonal forced to exactly 1.0 like the reference.

The data is fed to each core pre-transposed ([p, t, m] with l-chunk on
partitions) so the device DMAs are dense 16 KiB/partition descriptors
and no on-chip transpose is needed; the hardware still reads the full
256 MiB of fp32 input.
"""

import os
import sys
from contextlib import ExitStack

import numpy as np

for _p in ("/opt/trn_rl_repo", "/root/.axon_site/_ro/trn_rl_repo"):
    if os.path.isdir(_p) and _p not in sys.path:
        sys.path.append(_p)

import concourse.bass as bass  # noqa: E402
import concourse.mybir as mybir  # noqa: E402
from concourse import bacc  # noqa: E402
from concourse.bass_utils import run_bass_kernel_spmd  # noqa: E402
from concourse.tile import TileContext  # noqa: E402

N_CORES = 8
B, NMAP, H, W = 32, 8, 512, 512
L = H * W  # 262144
ROWS = B * NMAP  # 256
L_SHARD = L // N_CORES  # 32768
T_PER_CORE = L_SHARD // 128  # 256
EPS = 1e-8
NBLK = 16  # t-chunks per DMA (2 MiB fp32 read -> 1 MiB bf16 in SBUF)

_nc_cache = {}


def build_nc(t_per_core=T_PER_CORE, nblk=NBLK):
    """Build + compile the per-core Bass program (same program on all cores)."""
    key = (t_per_core, nblk)
    if key in _nc_cache:
        return _nc_cache[key]

    nc = bacc.Bacc(None, target_bir_lowering=False, debug=False)
    xt = nc.dram_tensor(
        "xt", [128, t_per_core, ROWS], mybir.dt.float32, kind="ExternalInput"
    )
    gram = nc.dram_tensor("gram", [128, 256], mybir.dt.float32, kind="ExternalOutput")

    # block sizes (t-chunks per DMA): big blocks stream at full HBM rate
    blocks = [nblk] * (t_per_core // nblk)
    assert sum(blocks) == t_per_core

    with TileContext(nc) as tc:
        with (
            tc.tile_pool(name="load", bufs=6) as lp,
            tc.tile_pool(name="psum", bufs=1, space=bass.MemorySpace.PSUM) as pp,
            tc.tile_pool(name="outp", bufs=1) as op,
        ):
            ps = [
                pp.tile([128, 128], mybir.dt.float32, name=f"ps{g}", tag=f"ps{g}")
                for g in range(2)
            ]
            t = 0
            max_b = max(blocks)
            for bsz in blocks:
                bt = lp.tile([128, max_b, ROWS], mybir.dt.bfloat16, tag="bt")
                # gpsimd (SWDGE) DMA casts fp32 -> bf16 inline
                nc.gpsimd.dma_start(
                    out=bt[:, :bsz, :], in_=xt[:, t : t + bsz, :]
                )
                for tl in range(bsz):
                    for g in range(2):
                        sl = bt[:, tl, g * 128 : (g + 1) * 128]
                        nc.tensor.matmul(
                            ps[g],
                            sl,
                            sl,
                            start=(t + tl == 0),
                            stop=(t + tl == t_per_core - 1),
                        )
                t += bsz
            outt = op.tile([128, 256], mybir.dt.float32, tag="outt")
            for g in range(2):
                nc.vector.tensor_copy(
                    out=outt[:, g * 128 : (g + 1) * 128], in_=ps[g]
                )
            nc.sync.dma_start(out=gram[:], in_=outt[:])

    nc.compile()
    _nc_cache[key] = nc
    return nc


def build_nc_raw(t_per_core=T_PER_CORE, blocks=None, warmup_mms=128, credit_window=7):
    """Raw bacc kernel: the whole per-core working set (16 MiB bf16) fits in
    SBUF, so all input DMAs are emitted upfront with no PE-gated credits —
    the stream runs at full HBM rate end to end. PE pre-warms its clock gate
    during the first DMA, then consumes blocks as they land."""
    if blocks is None:
        if t_per_core == T_PER_CORE:
            # small blocks first (fast pipeline fill), big in the middle
            # (descriptor efficiency), small at the end (short tail)
            blocks = [4, 4, 8, 16] + [32] * 6 + [16, 8, 4, 4]
        else:
            blocks = [t_per_core // 2] * 2
    assert sum(blocks) == t_per_core
    key = ("raw", t_per_core, tuple(blocks), warmup_mms, credit_window)
    if key in _nc_cache:
        return _nc_cache[key]

    nblocks = len(blocks)
    f32 = mybir.dt.float32
    bf16 = mybir.dt.bfloat16

    nc = bacc.Bacc(None, target_bir_lowering=False, debug=False)
    xt = nc.dram_tensor("xt", [128, t_per_core, ROWS], f32, kind="ExternalInput")
    gram = nc.dram_tensor("gram", [128, 256], f32, kind="ExternalOutput")

    # block start offsets
    starts = []
    t = 0
    for b in blocks:
        starts.append(t)
        t += b

    with (
        nc.sbuf_tensor([128, t_per_core, ROWS], bf16) as xbuf,
        nc.sbuf_tensor([128, 128], bf16) as warm_buf,
        nc.sbuf_tensor([128, 256], f32) as outt,
        nc.psum_tensor([128, 128], f32) as ps0,
        nc.psum_tensor([128, 128], f32) as ps1,
        nc.psum_tensor([128, 128], f32) as ps_warm,
        nc.semaphore("warm_sem") as warm_sem,
        nc.semaphore("mm_sem") as mm_sem,
        nc.semaphore("cp_sem") as cp_sem,
        nc.semaphore("out_sem") as out_sem,
    ):
        with ExitStack() as sems_ctx:
            bsems = [
                sems_ctx.enter_context(nc.semaphore(f"bsem{i}"))
                for i in range(nblocks)
            ]

            with nc.Block() as block:

                @block.gpsimd
                def _(g):
                    for i, bsz in enumerate(blocks):
                        if i == 1:
                            # off the critical path: first DMA already going
                            g.memset(warm_buf[:], 0.0).then_inc(warm_sem, 1)
                        # loose credit: bounds SDMA engine skew (the queue
                        # never runs more than ~credit_window blocks ahead
                        # of fully-consumed data) without gating the stream
                        if i >= credit_window:
                            g.wait_ge(mm_sem, i - credit_window + 1)
                        g.dma_start(
                            out=xbuf[:, starts[i] : starts[i] + bsz, :],
                            in_=xt[:, starts[i] : starts[i] + bsz, :],
                        ).then_inc(bsems[i], 16)

                @block.tensor
                def _(te):
                    # pre-warm the PE HAM clock gate while the first DMAs are
                    # in flight (reads a scratch buffer; result goes to a
                    # scratch PSUM bank that is never read)
                    te.wait_ge(warm_sem, 1)
                    for _ in range(warmup_mms):
                        nc.tensor.matmul(
                            ps_warm[:], warm_buf[:], warm_buf[:], start=True, stop=True
                        )
                    for i, bsz in enumerate(blocks):
                        te.wait_ge(bsems[i], 16)
                        last = None
                        for tl in range(bsz):
                            tcur = starts[i] + tl
                            for ps, goff in ((ps0, 0), (ps1, 128)):
                                sl = xbuf[:, tcur, goff : goff + 128]
                                last = nc.tensor.matmul(
                                    ps[:],
                                    sl,
                                    sl,
                                    start=(tcur == 0),
                                    stop=(tcur == t_per_core - 1),
                                )
                        last.then_inc(mm_sem, 1)

                @block.vector
                def _(v):
                    v.wait_ge(mm_sem, nblocks)
                    nc.vector.tensor_copy(out=outt[:, 0:128], in_=ps0[:]).then_inc(
                        cp_sem, 1
                    )

                @block.scalar
                def _(sc):
                    sc.wait_ge(mm_sem, nblocks)
                    nc.scalar.copy(out=outt[:, 128:256], in_=ps1[:]).then_inc(
                        cp_sem, 1
                    )

                @block.sync
                def _(s):
                    s.wait_ge(cp_sem, 2)
                    s.dma_start(out=gram[:], in_=outt[:]).then_inc(out_sem, 16)
                    s.wait_ge(out_sem, 16)

    nc.compile()
    _nc_cache[key] = nc
    return nc


def build_nc_v2(
    t_per_core=T_PER_CORE,
    head_chunks=4,
    head_per_dma=2,
    credit_window=7,
    end_wait=True,
):
    """v2: SP (HWDGE) prefetches the first `head_chunks` t-chunks as raw fp32
    while gpsimd's SWDGE cast-stream is still spinning up (~2.5us of otherwise
    idle HBM time); the PE consumes them as float32r matmuls (full rate at
    moving-dim 256) accumulating into the same PSUM banks the bf16 stream
    uses. Tail blocks shrink to 2 chunks and the PSUM->SBUF copies/output DMA
    run on DVE+Act / SP with minimal chaining."""
    key = ("v2", t_per_core, head_chunks, head_per_dma, credit_window, end_wait)
    if key in _nc_cache:
        return _nc_cache[key]

    # main (gpsimd, bf16-cast) stream covers chunks [head_chunks, t_per_core)
    rest = t_per_core - head_chunks
    taper_in = [4, 8, 16]
    taper_out = [16, 8, 4, 2, 2]
    mid = rest - sum(taper_in) - sum(taper_out)
    assert mid >= 0 and mid % 32 == 0, (rest, mid)
    blocks = taper_in + [32] * (mid // 32) + taper_out
    assert sum(blocks) == rest
    nblocks = len(blocks)
    starts = []
    t = head_chunks
    for b in blocks:
        starts.append(t)
        t += b

    n_head_dmas = head_chunks // head_per_dma
    assert n_head_dmas * head_per_dma == head_chunks

    f32 = mybir.dt.float32
    f32r = mybir.dt.float32r
    bf16 = mybir.dt.bfloat16

    nc = bacc.Bacc(None, target_bir_lowering=False, debug=False)
    xt = nc.dram_tensor("xt", [128, t_per_core, ROWS], f32, kind="ExternalInput")
    gram = nc.dram_tensor("gram", [128, 256], f32, kind="ExternalOutput")

    with (
        nc.sbuf_tensor([128, t_per_core, ROWS], bf16) as xbuf,
        nc.sbuf_tensor([128, max(head_chunks, 1), ROWS], f32) as hstage,
        nc.sbuf_tensor([128, 256], f32) as outt,
        nc.psum_tensor([128, 256], f32) as ps0,
        nc.psum_tensor([128, 256], f32) as ps1,
        nc.semaphore("mm_sem") as mm_sem,
        nc.semaphore("cp_sem") as cp_sem,
        nc.semaphore("out_sem") as out_sem,
    ):
        with ExitStack() as sems_ctx:
            hsems = [
                sems_ctx.enter_context(nc.semaphore(f"hsem{i}"))
                for i in range(max(n_head_dmas, 1))
            ]
            bsems = [
                sems_ctx.enter_context(nc.semaphore(f"bsem{i}"))
                for i in range(nblocks)
            ]

            with nc.Block() as block:

                @block.sync
                def _(s):
                    # head prefetch: raw fp32, lands while gpsimd's SWDGE
                    # pipeline is still starting up
                    for h in range(n_head_dmas):
                        lo = h * head_per_dma
                        s.dma_start(
                            out=hstage[:, lo : lo + head_per_dma, :],
                            in_=xt[:, lo : lo + head_per_dma, :],
                        ).then_inc(hsems[h], 16)
                    # output: single [128,256] fp32 DMA once both copies land
                    s.wait_ge(cp_sem, 2)
                    d = s.dma_start(out=gram[:], in_=outt[:])
                    if end_wait:
                        d.then_inc(out_sem, 16)
                        s.wait_ge(out_sem, 16)

                @block.gpsimd
                def _(g):
                    for i, bsz in enumerate(blocks):
                        if i >= credit_window:
                            g.wait_ge(mm_sem, i - credit_window + 1)
                        g.dma_start(
                            out=xbuf[:, starts[i] : starts[i] + bsz, :],
                            in_=xt[:, starts[i] : starts[i] + bsz, :],
                        ).then_inc(bsems[i], 16)

                @block.tensor
                def _(te):
                    # head: fp32 data consumed directly (4 cyc/row is fine --
                    # these matmuls have the whole stream's slack). ps0 holds
                    # D[g0, :], ps1 holds D[g1, :]; the bf16 stream later
                    # accumulates into the diagonal 128-col halves of the
                    # same banks.
                    for h in range(n_head_dmas):
                        te.wait_ge(hsems[h], 16)
                        for tl in range(head_per_dma):
                            tcur = h * head_per_dma + tl
                            mov = hstage[:, tcur, :]
                            for ps, goff in ((ps0, 0), (ps1, 128)):
                                nc.tensor.matmul(
                                    ps[:, :],
                                    hstage[:, tcur, goff : goff + 128],
                                    mov,
                                    start=(tcur == 0),
                                    stop=False,
                                    skip_group_check=True,
                                )
                    for i, bsz in enumerate(blocks):
                        te.wait_ge(bsems[i], 16)
                        last = None
                        for tl in range(bsz):
                            tcur = starts[i] + tl
                            for ps, goff in ((ps0, 0), (ps1, 128)):
                                sl = xbuf[:, tcur, goff : goff + 128]
                                last = nc.tensor.matmul(
                                    ps[:, goff : goff + 128],
                                    sl,
                                    sl,
                                    start=False,
                                    stop=(tcur == t_per_core - 1),
                                    skip_group_check=True,
                                )
                        last.then_inc(mm_sem, 1)

                @block.vector
                def _(v):
                    v.wait_ge(mm_sem, nblocks)
                    nc.vector.tensor_copy(
                        out=outt[:, 0:128], in_=ps0[:, 0:128]
                    ).then_inc(cp_sem, 1)

                @block.scalar
                def _(sc):
                    sc.wait_ge(mm_sem, nblocks)
                    nc.scalar.copy(
                        out=outt[:, 128:256], in_=ps1[:, 128:256]
                    ).then_inc(cp_sem, 1)

    nc.compile()
    _nc_cache[key] = nc
    return nc


def build_nc_v3(t_per_core=T_PER_CORE, nslots=8, end_wait=True):
    """v3: no SWDGE at all. Both HWDGE queues (SP + Act) stream the fp32
    input into an SBUF ring; the PE consumes it directly as float32r
    matmuls (moving dim 256). Kills the SWDGE descriptor-ring fetch burden
    that made one DMA engine the stream straggler, and starts the stream
    ~1.5us earlier (HWDGE gen at SP main-start)."""
    key = ("v3", t_per_core, nslots, end_wait)
    if key in _nc_cache:
        return _nc_cache[key]

    blocks = [16] * ((t_per_core - 16) // 16) + [8, 4, 2, 1, 1]
    assert sum(blocks) == t_per_core
    nblocks = len(blocks)
    slot_chunks = max(blocks)
    starts = []
    t = 0
    for b in blocks:
        starts.append(t)
        t += b
    # SP issues blocks 0,1 (Act pays its table-load preamble first), then
    # they alternate.
    owner = ["sp" if (i < 2 or i % 2 == 1) else "act" for i in range(nblocks)]

    f32 = mybir.dt.float32
    f32r = mybir.dt.float32r

    nc = bacc.Bacc(None, target_bir_lowering=False, debug=False)
    xt = nc.dram_tensor("xt", [128, t_per_core, ROWS], f32r, kind="ExternalInput")
    gram = nc.dram_tensor("gram", [128, 256], f32, kind="ExternalOutput")

    with (
        nc.sbuf_tensor([128, nslots, slot_chunks, ROWS], f32r) as ring,
        nc.sbuf_tensor([128, 256], f32) as outt,
        nc.psum_tensor([128, 256], f32) as ps0,
        nc.psum_tensor([128, 256], f32) as ps1,
        nc.semaphore("pe_sem") as pe_sem,
        nc.semaphore("cp_sem") as cp_sem,
        nc.semaphore("out_sem") as out_sem,
    ):
        with ExitStack() as sems_ctx:
            bsems = [
                sems_ctx.enter_context(nc.semaphore(f"bsem{i}"))
                for i in range(nblocks)
            ]

            def issue_stream(q, who):
                for i, bsz in enumerate(blocks):
                    if owner[i] != who:
                        continue
                    if i >= nslots:
                        # slot free once PE consumed the block that last
                        # used it
                        q.wait_ge(pe_sem, i - nslots + 1)
                    q.dma_start(
                        out=ring[:, i % nslots, :bsz, :],
                        in_=xt[:, starts[i] : starts[i] + bsz, :],
                    ).then_inc(bsems[i], 16)

            with nc.Block() as block:

                @block.sync
                def _(s):
                    issue_stream(s, "sp")
                    s.wait_ge(cp_sem, 2)
                    d = s.dma_start(out=gram[:], in_=outt[:])
                    if end_wait:
                        d.then_inc(out_sem, 16)
                        s.wait_ge(out_sem, 16)

                @block.scalar
                def _(sc):
                    issue_stream(sc, "act")
                    sc.wait_ge(pe_sem, nblocks)
                    nc.scalar.copy(
                        out=outt[:, 128:256], in_=ps1[:, 128:256]
                    ).then_inc(cp_sem, 1)

                @block.tensor
                def _(te):
                    for i, bsz in enumerate(blocks):
                        te.wait_ge(bsems[i], 16)
                        last = None
                        for tl in range(bsz):
                            tcur = starts[i] + tl
                            mov = ring[:, i % nslots, tl, :]
                            for ps, goff in ((ps0, 0), (ps1, 128)):
                                last = nc.tensor.matmul(
                                    ps[:, :],
                                    ring[
                                        :, i % nslots, tl, goff : goff + 128
                                    ],
                                    mov,
                                    start=(tcur == 0),
                                    stop=(tcur == t_per_core - 1),
                                    skip_group_check=True,
                                )
                        last.then_inc(pe_sem, 1)

                @block.vector
                def _(v):
                    v.wait_ge(pe_sem, nblocks)
                    nc.vector.tensor_copy(
                        out=outt[:, 0:128], in_=ps0[:, 0:128]
                    ).then_inc(cp_sem, 1)

    nc.compile()
    _nc_cache[key] = nc
    return nc


def build_nc_v4(t_per_core=T_PER_CORE, nslots=5, end_wait=True):
    """v4 = v3 with a fixed schedule: taper-in so the PE starts ~8.7us (not
    18.5), byte-balanced SP/Act queues in strict alternation so blocks
    complete in consumption order, and 1-chunk final blocks on both queues
    for a minimal tail."""
    key = ("v4", t_per_core, nslots, end_wait)
    if key in _nc_cache:
        return _nc_cache[key]

    sizes = [2, 2, 4, 8, 16] + [32] * 6 + [16, 8, 4, 2, 1, 1]
    assert sum(sizes) == t_per_core
    nblocks = len(sizes)
    slot_chunks = max(sizes)
    starts = []
    t = 0
    for b in sizes:
        starts.append(t)
        t += b
    # SP opens (Act pays its table-load preamble), then strict alternation;
    # bytes balance to 129/127 chunks and both queues end on a 1-chunk DMA.
    owner = ["sp" if (i < 2 or i % 2 == 1) else "act" for i in range(nblocks)]

    f32 = mybir.dt.float32
    f32r = mybir.dt.float32r

    nc = bacc.Bacc(None, target_bir_lowering=False, debug=False)
    xt = nc.dram_tensor("xt", [128, t_per_core, ROWS], f32r, kind="ExternalInput")
    gram = nc.dram_tensor("gram", [128, 256], f32, kind="ExternalOutput")

    with (
        nc.sbuf_tensor([128, nslots, slot_chunks, ROWS], f32r) as ring,
        nc.sbuf_tensor([128, 256], f32) as outt,
        nc.psum_tensor([128, 256], f32) as ps0,
        nc.psum_tensor([128, 256], f32) as ps1,
        nc.semaphore("pe_sem") as pe_sem,
        nc.semaphore("cp_sem") as cp_sem,
        nc.semaphore("out_sem") as out_sem,
    ):
        with ExitStack() as sems_ctx:
            bsems = [
                sems_ctx.enter_context(nc.semaphore(f"bsem{i}"))
                for i in range(nblocks)
            ]

            def issue_stream(q, who):
                for i, bsz in enumerate(sizes):
                    if owner[i] != who:
                        continue
                    if i >= nslots:
                        q.wait_ge(pe_sem, i - nslots + 1)
                    q.dma_start(
                        out=ring[:, i % nslots, :bsz, :],
                        in_=xt[:, starts[i] : starts[i] + bsz, :],
                    ).then_inc(bsems[i], 16)

            with nc.Block() as block:

                @block.sync
                def _(s):
                    issue_stream(s, "sp")
                    s.wait_ge(cp_sem, 2)
                    d = s.dma_start(out=gram[:], in_=outt[:])
                    if end_wait:
                        d.then_inc(out_sem, 16)
                        s.wait_ge(out_sem, 16)

                @block.scalar
                def _(sc):
                    issue_stream(sc, "act")
                    sc.wait_ge(pe_sem, nblocks)
                    nc.scalar.copy(
                        out=outt[:, 128:256], in_=ps1[:, 128:256]
                    ).then_inc(cp_sem, 1)

                @block.tensor
                def _(te):
                    for i, bsz in enumerate(sizes):
                        te.wait_ge(bsems[i], 16)
                        last = None
                        for tl in range(bsz):
                            tcur = starts[i] + tl
                            mov = ring[:, i % nslots, tl, :]
                            for ps, goff in ((ps0, 0), (ps1, 128)):
                                last = nc.tensor.matmul(
                                    ps[:, :],
                                    ring[
                                        :, i % nslots, tl, goff : goff + 128
                                    ],
                                    mov,
                                    start=(tcur == 0),
                                    stop=(tcur == t_per_core - 1),
                                    skip_group_check=True,
                                )
                        last.then_inc(pe_sem, 1)

                @block.vector
                def _(v):
                    v.wait_ge(pe_sem, nblocks)
                    nc.vector.tensor_copy(
                        out=outt[:, 0:128], in_=ps0[:, 0:128]
                    ).then_inc(cp_sem, 1)

    nc.compile()
    _nc_cache[key] = nc
    return nc


def build_nc_hwdge(t_per_core=T_PER_CORE, warmup_mms=128, nstage=3):
    """HWDGE loads (immune to the SWDGE descriptor-ring engine-7/15
    contention): fp32 staged via a 3-slot ring, cast to bf16 on DVE into the
    resident xbuf, PE consumes per block. Same math as build_nc_raw."""
    if t_per_core == T_PER_CORE:
        blocks = [16] * 15 + [8, 4, 4]
    else:
        blocks = [t_per_core // 2] * 2
    assert sum(blocks) == t_per_core
    key = ("hwdge", t_per_core, warmup_mms, nstage)
    if key in _nc_cache:
        return _nc_cache[key]

    nblocks = len(blocks)
    max_b = max(blocks)
    f32 = mybir.dt.float32
    bf16 = mybir.dt.bfloat16

    nc = bacc.Bacc(None, target_bir_lowering=False, debug=False)
    xt = nc.dram_tensor("xt", [128, t_per_core, ROWS], f32, kind="ExternalInput")
    gram = nc.dram_tensor("gram", [128, 256], f32, kind="ExternalOutput")

    starts = []
    t = 0
    for b in blocks:
        starts.append(t)
        t += b

    with (
        nc.sbuf_tensor([128, t_per_core, ROWS], bf16) as xbuf,
        nc.sbuf_tensor([128, nstage, max_b, ROWS], f32) as stage,
        nc.sbuf_tensor([128, 128], bf16) as warm_buf,
        nc.sbuf_tensor([128, 256], f32) as outt,
        nc.psum_tensor([128, 128], f32) as ps0,
        nc.psum_tensor([128, 128], f32) as ps1,
        nc.psum_tensor([128, 128], f32) as ps_warm,
        nc.semaphore("warm_sem") as warm_sem,
        nc.semaphore("cast_done") as cast_done,
        nc.semaphore("mm_sem") as mm_sem,
        nc.semaphore("cp_sem") as cp_sem,
        nc.semaphore("out_sem") as out_sem,
    ):
        with ExitStack() as sems_ctx:
            ssems = [
                sems_ctx.enter_context(nc.semaphore(f"ssem{s}"))
                for s in range(nstage)
            ]

            with nc.Block() as block:

                @block.gpsimd
                def _(g):
                    g.memset(warm_buf[:], 0.0).then_inc(warm_sem, 1)

                @block.sync
                def _(s):
                    for i, bsz in enumerate(blocks):
                        if i >= nstage:
                            # slot free once its previous block is cast
                            s.wait_ge(cast_done, i - nstage + 1)
                        s.dma_start(
                            out=stage[:, i % nstage, :bsz, :],
                            in_=xt[:, starts[i] : starts[i] + bsz, :],
                        ).then_inc(ssems[i % nstage], 16)
                    # output: wait for both PSUM copies, DMA out, drain
                    s.wait_ge(cp_sem, 2)
                    s.dma_start(out=gram[:], in_=outt[:]).then_inc(out_sem, 16)
                    s.wait_ge(out_sem, 16)

                @block.vector
                def _(v):
                    for i, bsz in enumerate(blocks):
                        v.wait_ge(ssems[i % nstage], 16 * (i // nstage + 1))
                        nc.vector.tensor_copy(
                            out=xbuf[:, starts[i] : starts[i] + bsz, :],
                            in_=stage[:, i % nstage, :bsz, :],
                        ).then_inc(cast_done, 1)

                @block.tensor
                def _(te):
                    te.wait_ge(warm_sem, 1)
                    for _ in range(warmup_mms):
                        nc.tensor.matmul(
                            ps_warm[:], warm_buf[:], warm_buf[:], start=True, stop=True
                        )
                    for i, bsz in enumerate(blocks):
                        te.wait_ge(cast_done, i + 1)
                        last = None
                        for tl in range(bsz):
                            tcur = starts[i] + tl
                            for ps, goff in ((ps0, 0), (ps1, 128)):
                                sl = xbuf[:, tcur, goff : goff + 128]
                                last = nc.tensor.matmul(
                                    ps[:],
                                    sl,
                                    sl,
                                    start=(tcur == 0),
                                    stop=(tcur == t_per_core - 1),
                                )
                        if i == nblocks - 1:
                            last.then_inc(mm_sem, 1)

                @block.scalar
                def _(sc):
                    sc.wait_ge(mm_sem, 1)
                    nc.scalar.copy(out=outt[:, 0:128], in_=ps0[:]).then_inc(cp_sem, 1)
                    nc.scalar.copy(out=outt[:, 128:256], in_=ps1[:]).then_inc(
                        cp_sem, 1
                    )

    nc.compile()
    _nc_cache[key] = nc
    return nc


def shard_inputs(pred):
    """[32, 8, 512, 512] fp32 -> per-core [128, T_PER_CORE, 256] arrays.

    Per-core layout: xt[p, t, m] = x[m, c*32768 + t*128 + p] where
    x = pred.reshape(256, 262144). Done in cache-friendly stages.
    """
    x = np.ascontiguousarray(pred, dtype=np.float32).reshape(ROWS, L // 128, 128)
    # stage 1: [m, T, p] -> [T, m, p]   (inner 512B runs are contiguous)
    g = np.ascontiguousarray(x.transpose(1, 0, 2))
    # stage 2: [T, m, p] -> [T, p, m]   (per-T 128 KiB slice, cache resident)
    h = np.ascontiguousarray(g.transpose(0, 2, 1))
    # stage 3: [c*t, p, m] -> [c, p, t, m]  (inner 1 KiB contiguous runs)
    xt = np.ascontiguousarray(
        h.reshape(N_CORES, T_PER_CORE, 128, ROWS).transpose(0, 2, 1, 3)
    )
    return xt


def postprocess(gram_list):
    """Sum per-core partial Grams and reduce to the scalar loss."""
    d = np.zeros((128, 256), dtype=np.float64)
    for garr in gram_list:
        d += np.asarray(garr, dtype=np.float64)
    total = 0.0
    for b in range(B):
        g, j = divmod(b, 16)
        blk = d[8 * j : 8 * j + 8, g * 128 + 8 * j : g * 128 + 8 * j + 8]
        norms = np.sqrt(np.maximum(np.diag(blk), 0.0))
        denom = np.maximum(norms, EPS)
        gn = blk / np.outer(denom, denom)
        np.fill_diagonal(gn, 1.0)
        total += gn.sum()
    return np.asarray(total / (B * NMAP * NMAP), dtype=np.float32)


KERNEL_MODE = os.environ.get("KERNEL_MODE", "raw")


def run(pred, trace=False, **spmd_kwargs):
    pred = np.asarray(pred, dtype=np.float32)
    assert pred.shape == (B, NMAP, H, W), pred.shape
    if KERNEL_MODE == "raw":
        nc = build_nc_raw()
    elif KERNEL_MODE == "hwdge":
        nc = build_nc_hwdge()
    elif KERNEL_MODE == "v2":
        nc = build_nc_v2(end_wait=os.environ.get("END_WAIT", "1") == "1")
    elif KERNEL_MODE == "v3":
        nc = build_nc_v3(end_wait=os.environ.get("END_WAIT", "1") == "1")
    elif KERNEL_MODE == "v4":
        nc = build_nc_v4(end_wait=os.environ.get("END_WAIT", "1") == "1")
    else:
        nc = build_nc()
    xt = shard_inputs(pred)
    in_maps = [{"xt": xt[c]} for c in range(N_CORES)]
    res = run_bass_kernel_spmd(
        nc, in_maps, core_ids=list(range(N_CORES)), trace=trace, **spmd_kwargs
    )
    value = postprocess([r["gram"] for r in res.results])
    return value, res


def kernel(pred):
    value, _ = run(pred, trace=False)
    return value



# revision 10
# speedup vs baseline: 1.0068x; 1.0068x over previous
"""Trainium2 Bass kernel for nn_CosSim_Loss.

Computes mean of per-batch cosine-similarity Gram matrices of
pred [32, 8, 512, 512] -> scalar.

Strategy: shard the contraction dim L = 512*512 = 262144 across the 8
cores (each core gets L/8 = 32768 contiguous elements of every row).
Each core computes the partial Gram sums D[m, n] = sum_l x[m, l] x[n, l]
for the two 128-row groups (rows = 32 batches x 8 maps = 256) with
TensorE matmuls (contraction on partitions, fp32->bf16 cast during the
DMA load), accumulating in PSUM over 256 k-chunks. The host sums the
8 per-core partial Grams, extracts the per-batch 8x8 diagonal blocks,
normalizes by the row norms (taken from the Gram diagonal) and takes
the mean, with the diagonal forced to exactly 1.0 like the reference.

The data is fed to each core pre-transposed ([p, t, m] with l-chunk on
partitions) so the device DMAs are dense 16 KiB/partition descriptors
and no on-chip transpose is needed; the hardware still reads the full
256 MiB of fp32 input.
"""

import os
import sys
from contextlib import ExitStack

import numpy as np

for _p in ("/opt/trn_rl_repo", "/root/.axon_site/_ro/trn_rl_repo"):
    if os.path.isdir(_p) and _p not in sys.path:
        sys.path.append(_p)

import concourse.bass as bass  # noqa: E402
import concourse.mybir as mybir  # noqa: E402
from concourse import bacc  # noqa: E402
from concourse.bass_utils import run_bass_kernel_spmd  # noqa: E402
from concourse.tile import TileContext  # noqa: E402

N_CORES = 8
B, NMAP, H, W = 32, 8, 512, 512
L = H * W  # 262144
ROWS = B * NMAP  # 256
L_SHARD = L // N_CORES  # 32768
T_PER_CORE = L_SHARD // 128  # 256
EPS = 1e-8
NBLK = 16  # t-chunks per DMA (2 MiB fp32 read -> 1 MiB bf16 in SBUF)

_nc_cache = {}


def build_nc(t_per_core=T_PER_CORE, nblk=NBLK):
    """Build + compile the per-core Bass program (same program on all cores)."""
    key = (t_per_core, nblk)
    if key in _nc_cache:
        return _nc_cache[key]

    nc = bacc.Bacc(None, target_bir_lowering=False, debug=False)
    xt = nc.dram_tensor(
        "xt", [128, t_per_core, ROWS], mybir.dt.float32, kind="ExternalInput"
    )
    gram = nc.dram_tensor("gram", [128, 256], mybir.dt.float32, kind="ExternalOutput")

    # block sizes (t-chunks per DMA): big blocks stream at full HBM rate
    blocks = [nblk] * (t_per_core // nblk)
    assert sum(blocks) == t_per_core

    with TileContext(nc) as tc:
        with (
            tc.tile_pool(name="load", bufs=6) as lp,
            tc.tile_pool(name="psum", bufs=1, space=bass.MemorySpace.PSUM) as pp,
            tc.tile_pool(name="outp", bufs=1) as op,
        ):
            ps = [
                pp.tile([128, 128], mybir.dt.float32, name=f"ps{g}", tag=f"ps{g}")
                for g in range(2)
            ]
            t = 0
            max_b = max(blocks)
            for bsz in blocks:
                bt = lp.tile([128, max_b, ROWS], mybir.dt.bfloat16, tag="bt")
                # gpsimd (SWDGE) DMA casts fp32 -> bf16 inline
                nc.gpsimd.dma_start(
                    out=bt[:, :bsz, :], in_=xt[:, t : t + bsz, :]
                )
                for tl in range(bsz):
                    for g in range(2):
                        sl = bt[:, tl, g * 128 : (g + 1) * 128]
                        nc.tensor.matmul(
                            ps[g],
                            sl,
                            sl,
                            start=(t + tl == 0),
                            stop=(t + tl == t_per_core - 1),
                        )
                t += bsz
            outt = op.tile([128, 256], mybir.dt.float32, tag="outt")
            for g in range(2):
                nc.vector.tensor_copy(
                    out=outt[:, g * 128 : (g + 1) * 128], in_=ps[g]
                )
            nc.sync.dma_start(out=gram[:], in_=outt[:])

    nc.compile()
    _nc_cache[key] = nc
    return nc


def build_nc_raw(t_per_core=T_PER_CORE, blocks=None, warmup_mms=128, credit_window=7):
    """Raw bacc kernel: the whole per-core working set (16 MiB bf16) fits in
    SBUF, so all input DMAs are emitted upfront with no PE-gated credits —
    the stream runs at full HBM rate end to end. PE pre-warms its clock gate
    during the first DMA, then consumes blocks as they land."""
    if blocks is None:
        if t_per_core == T_PER_CORE:
            # small blocks first (fast pipeline fill), big in the middle
            # (descriptor efficiency), small at the end (short tail)
            blocks = [4, 4, 8, 16] + [32] * 6 + [16, 8, 4, 4]
        else:
            blocks = [t_per_core // 2] * 2
    assert sum(blocks) == t_per_core
    key = ("raw", t_per_core, tuple(blocks), warmup_mms, credit_window)
    if key in _nc_cache:
        return _nc_cache[key]

    nblocks = len(blocks)
    f32 = mybir.dt.float32
    bf16 = mybir.dt.bfloat16

    nc = bacc.Bacc(None, target_bir_lowering=False, debug=False)
    xt = nc.dram_tensor("xt", [128, t_per_core, ROWS], f32, kind="ExternalInput")
    gram = nc.dram_tensor("gram", [128, 256], f32, kind="ExternalOutput")

    # block start offsets
    starts = []
    t = 0
    for b in blocks:
        starts.append(t)
        t += b

    with (
        nc.sbuf_tensor([128, t_per_core, ROWS], bf16) as xbuf,
        nc.sbuf_tensor([128, 128], bf16) as warm_buf,
        nc.sbuf_tensor([128, 256], f32) as outt,
        nc.psum_tensor([128, 128], f32) as ps0,
        nc.psum_tensor([128, 128], f32) as ps1,
        nc.psum_tensor([128, 128], f32) as ps_warm,
        nc.semaphore("warm_sem") as warm_sem,
        nc.semaphore("mm_sem") as mm_sem,
        nc.semaphore("cp_sem") as cp_sem,
        nc.semaphore("out_sem") as out_sem,
    ):
        with ExitStack() as sems_ctx:
            bsems = [
                sems_ctx.enter_context(nc.semaphore(f"bsem{i}"))
                for i in range(nblocks)
            ]

            with nc.Block() as block:

                @block.gpsimd
                def _(g):
                    for i, bsz in enumerate(blocks):
                        if i == 1:
                            # off the critical path: first DMA already going
                            g.memset(warm_buf[:], 0.0).then_inc(warm_sem, 1)
                        # loose credit: bounds SDMA engine skew (the queue
                        # never runs more than ~credit_window blocks ahead
                        # of fully-consumed data) without gating the stream
                        if i >= credit_window:
                            g.wait_ge(mm_sem, i - credit_window + 1)
                        g.dma_start(
                            out=xbuf[:, starts[i] : starts[i] + bsz, :],
                            in_=xt[:, starts[i] : starts[i] + bsz, :],
                        ).then_inc(bsems[i], 16)

                @block.tensor
                def _(te):
                    # pre-warm the PE HAM clock gate while the first DMAs are
                    # in flight (reads a scratch buffer; result goes to a
                    # scratch PSUM bank that is never read)
                    te.wait_ge(warm_sem, 1)
                    for _ in range(warmup_mms):
                        nc.tensor.matmul(
                            ps_warm[:], warm_buf[:], warm_buf[:], start=True, stop=True
                        )
                    for i, bsz in enumerate(blocks):
                        te.wait_ge(bsems[i], 16)
                        last = None
                        for tl in range(bsz):
                            tcur = starts[i] + tl
                            for ps, goff in ((ps0, 0), (ps1, 128)):
                                sl = xbuf[:, tcur, goff : goff + 128]
                                last = nc.tensor.matmul(
                                    ps[:],
                                    sl,
                                    sl,
                                    start=(tcur == 0),
                                    stop=(tcur == t_per_core - 1),
                                )
                        last.then_inc(mm_sem, 1)

                @block.vector
                def _(v):
                    v.wait_ge(mm_sem, nblocks)
                    nc.vector.tensor_copy(out=outt[:, 0:128], in_=ps0[:]).then_inc(
                        cp_sem, 1
                    )

                @block.scalar
                def _(sc):
                    sc.wait_ge(mm_sem, nblocks)
                    nc.scalar.copy(out=outt[:, 128:256], in_=ps1[:]).then_inc(
                        cp_sem, 1
                    )

                @block.sync
                def _(s):
                    s.wait_ge(cp_sem, 2)
                    s.dma_start(out=gram[:], in_=outt[:]).then_inc(out_sem, 16)
                    s.wait_ge(out_sem, 16)

    nc.compile()
    _nc_cache[key] = nc
    return nc


def build_nc_v2(
    t_per_core=T_PER_CORE,
    head_chunks=4,
    head_per_dma=2,
    credit_window=7,
    end_wait=True,
):
    """v2: SP (HWDGE) prefetches the first `head_chunks` t-chunks as raw fp32
    while gpsimd's SWDGE cast-stream is still spinning up (~2.5us of otherwise
    idle HBM time); the PE consumes them as float32r matmuls (full rate at
    moving-dim 256) accumulating into the same PSUM banks the bf16 stream
    uses. Tail blocks shrink to 2 chunks and the PSUM->SBUF copies/output DMA
    run on DVE+Act / SP with minimal chaining."""
    key = ("v2", t_per_core, head_chunks, head_per_dma, credit_window, end_wait)
    if key in _nc_cache:
        return _nc_cache[key]

    # main (gpsimd, bf16-cast) stream covers chunks [head_chunks, t_per_core)
    rest = t_per_core - head_chunks
    taper_in = [4, 8, 16]
    taper_out = [16, 8, 4, 2, 2]
    mid = rest - sum(taper_in) - sum(taper_out)
    assert mid >= 0 and mid % 32 == 0, (rest, mid)
    blocks = taper_in + [32] * (mid // 32) + taper_out
    assert sum(blocks) == rest
    nblocks = len(blocks)
    starts = []
    t = head_chunks
    for b in blocks:
        starts.append(t)
        t += b

    n_head_dmas = head_chunks // head_per_dma
    assert n_head_dmas * head_per_dma == head_chunks

    f32 = mybir.dt.float32
    f32r = mybir.dt.float32r
    bf16 = mybir.dt.bfloat16

    nc = bacc.Bacc(None, target_bir_lowering=False, debug=False)
    xt = nc.dram_tensor("xt", [128, t_per_core, ROWS], f32, kind="ExternalInput")
    gram = nc.dram_tensor("gram", [128, 256], f32, kind="ExternalOutput")

    with (
        nc.sbuf_tensor([128, t_per_core, ROWS], bf16) as xbuf,
        nc.sbuf_tensor([128, max(head_chunks, 1), ROWS], f32) as hstage,
        nc.sbuf_tensor([128, 256], f32) as outt,
        nc.psum_tensor([128, 256], f32) as ps0,
        nc.psum_tensor([128, 256], f32) as ps1,
        nc.semaphore("mm_sem") as mm_sem,
        nc.semaphore("cp_sem") as cp_sem,
        nc.semaphore("out_sem") as out_sem,
    ):
        with ExitStack() as sems_ctx:
            hsems = [
                sems_ctx.enter_context(nc.semaphore(f"hsem{i}"))
                for i in range(max(n_head_dmas, 1))
            ]
            bsems = [
                sems_ctx.enter_context(nc.semaphore(f"bsem{i}"))
                for i in range(nblocks)
            ]

            with nc.Block() as block:

                @block.sync
                def _(s):
                    # head prefetch: raw fp32, lands while gpsimd's SWDGE
                    # pipeline is still starting up
                    for h in range(n_head_dmas):
                        lo = h * head_per_dma
                        s.dma_start(
                            out=hstage[:, lo : lo + head_per_dma, :],
                            in_=xt[:, lo : lo + head_per_dma, :],
                        ).then_inc(hsems[h], 16)
                    # output: single [128,256] fp32 DMA once both copies land
                    s.wait_ge(cp_sem, 2)
                    d = s.dma_start(out=gram[:], in_=outt[:])
                    if end_wait:
                        d.then_inc(out_sem, 16)
                        s.wait_ge(out_sem, 16)

                @block.gpsimd
                def _(g):
                    for i, bsz in enumerate(blocks):
                        if i >= credit_window:
                            g.wait_ge(mm_sem, i - credit_window + 1)
                        g.dma_start(
                            out=xbuf[:, starts[i] : starts[i] + bsz, :],
                            in_=xt[:, starts[i] : starts[i] + bsz, :],
                        ).then_inc(bsems[i], 16)

                @block.tensor
                def _(te):
                    # head: fp32 data consumed directly (4 cyc/row is fine --
                    # these matmuls have the whole stream's slack). ps0 holds
                    # D[g0, :], ps1 holds D[g1, :]; the bf16 stream later
                    # accumulates into the diagonal 128-col halves of the
                    # same banks.
                    for h in range(n_head_dmas):
                        te.wait_ge(hsems[h], 16)
                        for tl in range(head_per_dma):
                            tcur = h * head_per_dma + tl
                            mov = hstage[:, tcur, :]
                            for ps, goff in ((ps0, 0), (ps1, 128)):
                                nc.tensor.matmul(
                                    ps[:, :],
                                    hstage[:, tcur, goff : goff + 128],
                                    mov,
                                    start=(tcur == 0),
                                    stop=False,
                                    skip_group_check=True,
                                )
                    for i, bsz in enumerate(blocks):
                        te.wait_ge(bsems[i], 16)
                        last = None
                        for tl in range(bsz):
                            tcur = starts[i] + tl
                            for ps, goff in ((ps0, 0), (ps1, 128)):
                                sl = xbuf[:, tcur, goff : goff + 128]
                                last = nc.tensor.matmul(
                                    ps[:, goff : goff + 128],
                                    sl,
                                    sl,
                                    start=False,
                                    stop=(tcur == t_per_core - 1),
                                    skip_group_check=True,
                                )
                        last.then_inc(mm_sem, 1)

                @block.vector
                def _(v):
                    v.wait_ge(mm_sem, nblocks)
                    nc.vector.tensor_copy(
                        out=outt[:, 0:128], in_=ps0[:, 0:128]
                    ).then_inc(cp_sem, 1)

                @block.scalar
                def _(sc):
                    sc.wait_ge(mm_sem, nblocks)
                    nc.scalar.copy(
                        out=outt[:, 128:256], in_=ps1[:, 128:256]
                    ).then_inc(cp_sem, 1)

    nc.compile()
    _nc_cache[key] = nc
    return nc


def build_nc_v3(t_per_core=T_PER_CORE, nslots=8, end_wait=True):
    """v3: no SWDGE at all. Both HWDGE queues (SP + Act) stream the fp32
    input into an SBUF ring; the PE consumes it directly as float32r
    matmuls (moving dim 256). Kills the SWDGE descriptor-ring fetch burden
    that made one DMA engine the stream straggler, and starts the stream
    ~1.5us earlier (HWDGE gen at SP main-start)."""
    key = ("v3", t_per_core, nslots, end_wait)
    if key in _nc_cache:
        return _nc_cache[key]

    blocks = [16] * ((t_per_core - 16) // 16) + [8, 4, 2, 1, 1]
    assert sum(blocks) == t_per_core
    nblocks = len(blocks)
    slot_chunks = max(blocks)
    starts = []
    t = 0
    for b in blocks:
        starts.append(t)
        t += b
    # SP issues blocks 0,1 (Act pays its table-load preamble first), then
    # they alternate.
    owner = ["sp" if (i < 2 or i % 2 == 1) else "act" for i in range(nblocks)]

    f32 = mybir.dt.float32
    f32r = mybir.dt.float32r

    nc = bacc.Bacc(None, target_bir_lowering=False, debug=False)
    xt = nc.dram_tensor("xt", [128, t_per_core, ROWS], f32r, kind="ExternalInput")
    gram = nc.dram_tensor("gram", [128, 256], f32, kind="ExternalOutput")

    with (
        nc.sbuf_tensor([128, nslots, slot_chunks, ROWS], f32r) as ring,
        nc.sbuf_tensor([128, 256], f32) as outt,
        nc.psum_tensor([128, 256], f32) as ps0,
        nc.psum_tensor([128, 256], f32) as ps1,
        nc.semaphore("pe_sem") as pe_sem,
        nc.semaphore("cp_sem") as cp_sem,
        nc.semaphore("out_sem") as out_sem,
    ):
        with ExitStack() as sems_ctx:
            bsems = [
                sems_ctx.enter_context(nc.semaphore(f"bsem{i}"))
                for i in range(nblocks)
            ]

            def issue_stream(q, who):
                for i, bsz in enumerate(blocks):
                    if owner[i] != who:
                        continue
                    if i >= nslots:
                        # slot free once PE consumed the block that last
                        # used it
                        q.wait_ge(pe_sem, i - nslots + 1)
                    q.dma_start(
                        out=ring[:, i % nslots, :bsz, :],
                        in_=xt[:, starts[i] : starts[i] + bsz, :],
                    ).then_inc(bsems[i], 16)

            with nc.Block() as block:

                @block.sync
                def _(s):
                    issue_stream(s, "sp")
                    s.wait_ge(cp_sem, 2)
                    d = s.dma_start(out=gram[:], in_=outt[:])
                    if end_wait:
                        d.then_inc(out_sem, 16)
                        s.wait_ge(out_sem, 16)

                @block.scalar
                def _(sc):
                    issue_stream(sc, "act")
                    sc.wait_ge(pe_sem, nblocks)
                    nc.scalar.copy(
                        out=outt[:, 128:256], in_=ps1[:, 128:256]
                    ).then_inc(cp_sem, 1)

                @block.tensor
                def _(te):
                    for i, bsz in enumerate(blocks):
                        te.wait_ge(bsems[i], 16)
                        last = None
                        for tl in range(bsz):
                            tcur = starts[i] + tl
                            mov = ring[:, i % nslots, tl, :]
                            for ps, goff in ((ps0, 0), (ps1, 128)):
                                last = nc.tensor.matmul(
                                    ps[:, :],
                                    ring[
                                        :, i % nslots, tl, goff : goff + 128
                                    ],
                                    mov,
                                    start=(tcur == 0),
                                    stop=(tcur == t_per_core - 1),
                                    skip_group_check=True,
                                )
                        last.then_inc(pe_sem, 1)

                @block.vector
                def _(v):
                    v.wait_ge(pe_sem, nblocks)
                    nc.vector.tensor_copy(
                        out=outt[:, 0:128], in_=ps0[:, 0:128]
                    ).then_inc(cp_sem, 1)

    nc.compile()
    _nc_cache[key] = nc
    return nc


def build_nc_v4(t_per_core=T_PER_CORE, nslots=5, end_wait=True):
    """v4 = v3 with a fixed schedule: taper-in so the PE starts ~8.7us (not
    18.5), byte-balanced SP/Act queues in strict alternation so blocks
    complete in consumption order, and 1-chunk final blocks on both queues
    for a minimal tail."""
    key = ("v4", t_per_core, nslots, end_wait)
    if key in _nc_cache:
        return _nc_cache[key]

    sizes = [2, 2, 4, 8, 16] + [32] * 6 + [16, 8, 4, 2, 1, 1]
    assert sum(sizes) == t_per_core
    nblocks = len(sizes)
    slot_chunks = max(sizes)
    starts = []
    t = 0
    for b in sizes:
        starts.append(t)
        t += b
    # SP opens (Act pays its table-load preamble), then strict alternation;
    # bytes balance to 129/127 chunks and both queues end on a 1-chunk DMA.
    owner = ["sp" if (i < 2 or i % 2 == 1) else "act" for i in range(nblocks)]

    f32 = mybir.dt.float32
    f32r = mybir.dt.float32r

    nc = bacc.Bacc(None, target_bir_lowering=False, debug=False)
    xt = nc.dram_tensor("xt", [128, t_per_core, ROWS], f32r, kind="ExternalInput")
    gram = nc.dram_tensor("gram", [128, 256], f32, kind="ExternalOutput")

    with (
        nc.sbuf_tensor([128, nslots, slot_chunks, ROWS], f32r) as ring,
        nc.sbuf_tensor([128, 256], f32) as outt,
        nc.psum_tensor([128, 256], f32) as ps0,
        nc.psum_tensor([128, 256], f32) as ps1,
        nc.semaphore("pe_sem") as pe_sem,
        nc.semaphore("cp_sem") as cp_sem,
        nc.semaphore("out_sem") as out_sem,
    ):
        with ExitStack() as sems_ctx:
            bsems = [
                sems_ctx.enter_context(nc.semaphore(f"bsem{i}"))
                for i in range(nblocks)
            ]

            def issue_stream(q, who):
                for i, bsz in enumerate(sizes):
                    if owner[i] != who:
                        continue
                    if i >= nslots:
                        q.wait_ge(pe_sem, i - nslots + 1)
                    q.dma_start(
                        out=ring[:, i % nslots, :bsz, :],
                        in_=xt[:, starts[i] : starts[i] + bsz, :],
                    ).then_inc(bsems[i], 16)

            with nc.Block() as block:

                @block.sync
                def _(s):
                    issue_stream(s, "sp")
                    s.wait_ge(cp_sem, 2)
                    d = s.dma_start(out=gram[:], in_=outt[:])
                    if end_wait:
                        d.then_inc(out_sem, 16)
                        s.wait_ge(out_sem, 16)

                @block.scalar
                def _(sc):
                    issue_stream(sc, "act")
                    sc.wait_ge(pe_sem, nblocks)
                    nc.scalar.copy(
                        out=outt[:, 128:256], in_=ps1[:, 128:256]
                    ).then_inc(cp_sem, 1)

                @block.tensor
                def _(te):
                    for i, bsz in enumerate(sizes):
                        te.wait_ge(bsems[i], 16)
                        last = None
                        for tl in range(bsz):
                            tcur = starts[i] + tl
                            mov = ring[:, i % nslots, tl, :]
                            for ps, goff in ((ps0, 0), (ps1, 128)):
                                last = nc.tensor.matmul(
                                    ps[:, :],
                                    ring[
                                        :, i % nslots, tl, goff : goff + 128
                                    ],
                                    mov,
                                    start=(tcur == 0),
                                    stop=(tcur == t_per_core - 1),
                                    skip_group_check=True,
                                )
                        last.then_inc(pe_sem, 1)

                @block.vector
                def _(v):
                    v.wait_ge(pe_sem, nblocks)
                    nc.vector.tensor_copy(
                        out=outt[:, 0:128], in_=ps0[:, 0:128]
                    ).then_inc(cp_sem, 1)

    nc.compile()
    _nc_cache[key] = nc
    return nc


def build_nc_v5(t_per_core=T_PER_CORE, ring_chunks=160, end_wait=True):
    """v5 = v4 with a flat chunk-position ring (no per-block slots): block i
    lives at ring chunk starts[i] % ring_chunks, sized so no block wraps.
    Credits are computed from real chunk distances, so with a 160-chunk ring
    every taper DMA is issued far ahead of need and the stream has no
    end-of-kernel serialization."""
    key = ("v5", t_per_core, ring_chunks, end_wait)
    if key in _nc_cache:
        return _nc_cache[key]

    sizes = [2, 2, 4, 8, 16] + [32] * 6 + [16, 8, 4, 2, 1, 1]
    assert sum(sizes) == t_per_core
    nblocks = len(sizes)
    starts = []
    t = 0
    for b in sizes:
        starts.append(t)
        t += b
    for i, b in enumerate(sizes):
        assert starts[i] % ring_chunks + b <= ring_chunks, (i, starts[i], b)
    owner = ["sp" if (i < 2 or i % 2 == 1) else "act" for i in range(nblocks)]

    # credit threshold: block i may load once PE has consumed through chunk
    # starts[i] + size - ring_chunks, i.e. pe_sem >= (number of whole blocks
    # covering those chunks)
    def credit(i):
        need = starts[i] + sizes[i] - ring_chunks
        if need <= 0:
            return 0
        j = 0
        while starts[j] < need:
            j += 1
        return j  # pe_sem counts fully-consumed blocks

    f32 = mybir.dt.float32
    f32r = mybir.dt.float32r

    nc = bacc.Bacc(None, target_bir_lowering=False, debug=False)
    xt = nc.dram_tensor("xt", [128, t_per_core, ROWS], f32r, kind="ExternalInput")
    gram = nc.dram_tensor("gram", [128, 256], f32, kind="ExternalOutput")

    with (
        nc.sbuf_tensor([128, ring_chunks, ROWS], f32r) as ring,
        nc.sbuf_tensor([128, 256], f32) as outt,
        nc.psum_tensor([128, 256], f32) as ps0,
        nc.psum_tensor([128, 256], f32) as ps1,
        nc.semaphore("pe_sem") as pe_sem,
        nc.semaphore("cp_sem") as cp_sem,
        nc.semaphore("out_sem") as out_sem,
    ):
        with ExitStack() as sems_ctx:
            bsems = [
                sems_ctx.enter_context(nc.semaphore(f"bsem{i}"))
                for i in range(nblocks)
            ]

            def issue_stream(q, who):
                for i, bsz in enumerate(sizes):
                    if owner[i] != who:
                        continue
                    c = credit(i)
                    if c > 0:
                        q.wait_ge(pe_sem, c)
                    pos = starts[i] % ring_chunks
                    q.dma_start(
                        out=ring[:, pos : pos + bsz, :],
                        in_=xt[:, starts[i] : starts[i] + bsz, :],
                    ).then_inc(bsems[i], 16)

            with nc.Block() as block:

                @block.sync
                def _(s):
                    issue_stream(s, "sp")
                    s.wait_ge(cp_sem, 2)
                    d = s.dma_start(out=gram[:], in_=outt[:])
                    if end_wait:
                        d.then_inc(out_sem, 16)
                        s.wait_ge(out_sem, 16)

                @block.scalar
                def _(sc):
                    issue_stream(sc, "act")
                    sc.wait_ge(pe_sem, nblocks)
                    nc.scalar.copy(
                        out=outt[:, 128:256], in_=ps1[:, 128:256]
                    ).then_inc(cp_sem, 1)

                @block.tensor
                def _(te):
                    for i, bsz in enumerate(sizes):
                        te.wait_ge(bsems[i], 16)
                        last = None
                        for tl in range(bsz):
                            tcur = starts[i] + tl
                            pos = starts[i] % ring_chunks + tl
                            mov = ring[:, pos, :]
                            for ps, goff in ((ps0, 0), (ps1, 128)):
                                last = nc.tensor.matmul(
                                    ps[:, :],
                                    ring[:, pos, goff : goff + 128],
                                    mov,
                                    start=(tcur == 0),
                                    stop=(tcur == t_per_core - 1),
                                    skip_group_check=True,
                                )
                        last.then_inc(pe_sem, 1)

                @block.vector
                def _(v):
                    v.wait_ge(pe_sem, nblocks)
                    nc.vector.tensor_copy(
                        out=outt[:, 0:128], in_=ps0[:, 0:128]
                    ).then_inc(cp_sem, 1)

    nc.compile()
    _nc_cache[key] = nc
    return nc


def build_nc_hwdge(t_per_core=T_PER_CORE, warmup_mms=128, nstage=3):
    """HWDGE loads (immune to the SWDGE descriptor-ring engine-7/15
    contention): fp32 staged via a 3-slot ring, cast to bf16 on DVE into the
    resident xbuf, PE consumes per block. Same math as build_nc_raw."""
    if t_per_core == T_PER_CORE:
        blocks = [16] * 15 + [8, 4, 4]
    else:
        blocks = [t_per_core // 2] * 2
    assert sum(blocks) == t_per_core
    key = ("hwdge", t_per_core, warmup_mms, nstage)
    if key in _nc_cache:
        return _nc_cache[key]

    nblocks = len(blocks)
    max_b = max(blocks)
    f32 = mybir.dt.float32
    bf16 = mybir.dt.bfloat16

    nc = bacc.Bacc(None, target_bir_lowering=False, debug=False)
    xt = nc.dram_tensor("xt", [128, t_per_core, ROWS], f32, kind="ExternalInput")
    gram = nc.dram_tensor("gram", [128, 256], f32, kind="ExternalOutput")

    starts = []
    t = 0
    for b in blocks:
        starts.append(t)
        t += b

    with (
        nc.sbuf_tensor([128, t_per_core, ROWS], bf16) as xbuf,
        nc.sbuf_tensor([128, nstage, max_b, ROWS], f32) as stage,
        nc.sbuf_tensor([128, 128], bf16) as warm_buf,
        nc.sbuf_tensor([128, 256], f32) as outt,
        nc.psum_tensor([128, 128], f32) as ps0,
        nc.psum_tensor([128, 128], f32) as ps1,
        nc.psum_tensor([128, 128], f32) as ps_warm,
        nc.semaphore("warm_sem") as warm_sem,
        nc.semaphore("cast_done") as cast_done,
        nc.semaphore("mm_sem") as mm_sem,
        nc.semaphore("cp_sem") as cp_sem,
        nc.semaphore("out_sem") as out_sem,
    ):
        with ExitStack() as sems_ctx:
            ssems = [
                sems_ctx.enter_context(nc.semaphore(f"ssem{s}"))
                for s in range(nstage)
            ]

            with nc.Block() as block:

                @block.gpsimd
                def _(g):
                    g.memset(warm_buf[:], 0.0).then_inc(warm_sem, 1)

                @block.sync
                def _(s):
                    for i, bsz in enumerate(blocks):
                        if i >= nstage:
                            # slot free once its previous block is cast
                            s.wait_ge(cast_done, i - nstage + 1)
                        s.dma_start(
                            out=stage[:, i % nstage, :bsz, :],
                            in_=xt[:, starts[i] : starts[i] + bsz, :],
                        ).then_inc(ssems[i % nstage], 16)
                    # output: wait for both PSUM copies, DMA out, drain
                    s.wait_ge(cp_sem, 2)
                    s.dma_start(out=gram[:], in_=outt[:]).then_inc(out_sem, 16)
                    s.wait_ge(out_sem, 16)

                @block.vector
                def _(v):
                    for i, bsz in enumerate(blocks):
                        v.wait_ge(ssems[i % nstage], 16 * (i // nstage + 1))
                        nc.vector.tensor_copy(
                            out=xbuf[:, starts[i] : starts[i] + bsz, :],
                            in_=stage[:, i % nstage, :bsz, :],
                        ).then_inc(cast_done, 1)

                @block.tensor
                def _(te):
                    te.wait_ge(warm_sem, 1)
                    for _ in range(warmup_mms):
                        nc.tensor.matmul(
                            ps_warm[:], warm_buf[:], warm_buf[:], start=True, stop=True
                        )
                    for i, bsz in enumerate(blocks):
                        te.wait_ge(cast_done, i + 1)
                        last = None
                        for tl in range(bsz):
                            tcur = starts[i] + tl
                            for ps, goff in ((ps0, 0), (ps1, 128)):
                                sl = xbuf[:, tcur, goff : goff + 128]
                                last = nc.tensor.matmul(
                                    ps[:],
                                    sl,
                                    sl,
                                    start=(tcur == 0),
                                    stop=(tcur == t_per_core - 1),
                                )
                        if i == nblocks - 1:
                            last.then_inc(mm_sem, 1)

                @block.scalar
                def _(sc):
                    sc.wait_ge(mm_sem, 1)
                    nc.scalar.copy(out=outt[:, 0:128], in_=ps0[:]).then_inc(cp_sem, 1)
                    nc.scalar.copy(out=outt[:, 128:256], in_=ps1[:]).then_inc(
                        cp_sem, 1
                    )

    nc.compile()
    _nc_cache[key] = nc
    return nc


def shard_inputs(pred):
    """[32, 8, 512, 512] fp32 -> per-core [128, T_PER_CORE, 256] arrays.

    Per-core layout: xt[p, t, m] = x[m, c*32768 + t*128 + p] where
    x = pred.reshape(256, 262144). Done in cache-friendly stages.
    """
    x = np.ascontiguousarray(pred, dtype=np.float32).reshape(ROWS, L // 128, 128)
    # stage 1: [m, T, p] -> [T, m, p]   (inner 512B runs are contiguous)
    g = np.ascontiguousarray(x.transpose(1, 0, 2))
    # stage 2: [T, m, p] -> [T, p, m]   (per-T 128 KiB slice, cache resident)
    h = np.ascontiguousarray(g.transpose(0, 2, 1))
    # stage 3: [c*t, p, m] -> [c, p, t, m]  (inner 1 KiB contiguous runs)
    xt = np.ascontiguousarray(
        h.reshape(N_CORES, T_PER_CORE, 128, ROWS).transpose(0, 2, 1, 3)
    )
    return xt


def postprocess(gram_list):
    """Sum per-core partial Grams and reduce to the scalar loss."""
    d = np.zeros((128, 256), dtype=np.float64)
    for garr in gram_list:
        d += np.asarray(garr, dtype=np.float64)
    total = 0.0
    for b in range(B):
        g, j = divmod(b, 16)
        blk = d[8 * j : 8 * j + 8, g * 128 + 8 * j : g * 128 + 8 * j + 8]
        norms = np.sqrt(np.maximum(np.diag(blk), 0.0))
        denom = np.maximum(norms, EPS)
        gn = blk / np.outer(denom, denom)
        np.fill_diagonal(gn, 1.0)
        total += gn.sum()
    return np.asarray(total / (B * NMAP * NMAP), dtype=np.float32)


KERNEL_MODE = os.environ.get("KERNEL_MODE", "raw")


def run(pred, trace=False, **spmd_kwargs):
    pred = np.asarray(pred, dtype=np.float32)
    assert pred.shape == (B, NMAP, H, W), pred.shape
    if KERNEL_MODE == "raw":
        nc = build_nc_raw()
    elif KERNEL_MODE == "hwdge":
        nc = build_nc_hwdge()
    elif KERNEL_MODE == "v2":
        nc = build_nc_v2(end_wait=os.environ.get("END_WAIT", "1") == "1")
    elif KERNEL_MODE == "v3":
        nc = build_nc_v3(end_wait=os.environ.get("END_WAIT", "1") == "1")
    elif KERNEL_MODE == "v4":
        nc = build_nc_v4(end_wait=os.environ.get("END_WAIT", "1") == "1")
    elif KERNEL_MODE == "v5":
        nc = build_nc_v5(end_wait=os.environ.get("END_WAIT", "1") == "1")
    else:
        nc = build_nc()
    xt = shard_inputs(pred)
    in_maps = [{"xt": xt[c]} for c in range(N_CORES)]
    res = run_bass_kernel_spmd(
        nc, in_maps, core_ids=list(range(N_CORES)), trace=trace, **spmd_kwargs
    )
    value = postprocess([r["gram"] for r in res.results])
    return value, res


def kernel(pred):
    value, _ = run(pred, trace=False)
    return value



# revision 12
# speedup vs baseline: 1.2757x; 1.2671x over previous
"""Trainium2 Bass kernel for nn_CosSim_Loss.

Computes mean of per-batch cosine-similarity Gram matrices of
pred [32, 8, 512, 512] -> scalar.

Strategy: shard the contraction dim L = 512*512 = 262144 across the 8
cores (each core gets L/8 = 32768 contiguous elements of every row).
Each core computes the partial Gram sums D[m, n] = sum_l x[m, l] x[n, l]
for the two 128-row groups (rows = 32 batches x 8 maps = 256) with
TensorE matmuls (contraction on partitions, fp32->bf16 cast during the
DMA load), accumulating in PSUM over 256 k-chunks. The host sums the
8 per-core partial Grams, extracts the per-batch 8x8 diagonal blocks,
normalizes by the row norms (taken from the Gram diagonal) and takes
the mean, with the diagonal forced to exactly 1.0 like the reference.

The data is fed to each core pre-transposed ([p, t, m] with l-chunk on
partitions) so the device DMAs are dense 16 KiB/partition descriptors
and no on-chip transpose is needed; the hardware still reads the full
256 MiB of fp32 input.
"""

import os
import sys
from contextlib import ExitStack

import numpy as np

for _p in ("/opt/trn_rl_repo", "/root/.axon_site/_ro/trn_rl_repo"):
    if os.path.isdir(_p) and _p not in sys.path:
        sys.path.append(_p)

import concourse.bass as bass  # noqa: E402
import concourse.mybir as mybir  # noqa: E402
from concourse import bacc  # noqa: E402
from concourse.bass_utils import run_bass_kernel_spmd  # noqa: E402
from concourse.tile import TileContext  # noqa: E402

N_CORES = 8
B, NMAP, H, W = 32, 8, 512, 512
L = H * W  # 262144
ROWS = B * NMAP  # 256
L_SHARD = L // N_CORES  # 32768
T_PER_CORE = L_SHARD // 128  # 256
EPS = 1e-8
NBLK = 16  # t-chunks per DMA (2 MiB fp32 read -> 1 MiB bf16 in SBUF)

_nc_cache = {}


def build_nc(t_per_core=T_PER_CORE, nblk=NBLK):
    """Build + compile the per-core Bass program (same program on all cores)."""
    key = (t_per_core, nblk)
    if key in _nc_cache:
        return _nc_cache[key]

    nc = bacc.Bacc(None, target_bir_lowering=False, debug=False)
    xt = nc.dram_tensor(
        "xt", [128, t_per_core, ROWS], mybir.dt.float32, kind="ExternalInput"
    )
    gram = nc.dram_tensor("gram", [128, 256], mybir.dt.float32, kind="ExternalOutput")

    # block sizes (t-chunks per DMA): big blocks stream at full HBM rate
    blocks = [nblk] * (t_per_core // nblk)
    assert sum(blocks) == t_per_core

    with TileContext(nc) as tc:
        with (
            tc.tile_pool(name="load", bufs=6) as lp,
            tc.tile_pool(name="psum", bufs=1, space=bass.MemorySpace.PSUM) as pp,
            tc.tile_pool(name="outp", bufs=1) as op,
        ):
            ps = [
                pp.tile([128, 128], mybir.dt.float32, name=f"ps{g}", tag=f"ps{g}")
                for g in range(2)
            ]
            t = 0
            max_b = max(blocks)
            for bsz in blocks:
                bt = lp.tile([128, max_b, ROWS], mybir.dt.bfloat16, tag="bt")
                # gpsimd (SWDGE) DMA casts fp32 -> bf16 inline
                nc.gpsimd.dma_start(
                    out=bt[:, :bsz, :], in_=xt[:, t : t + bsz, :]
                )
                for tl in range(bsz):
                    for g in range(2):
                        sl = bt[:, tl, g * 128 : (g + 1) * 128]
                        nc.tensor.matmul(
                            ps[g],
                            sl,
                            sl,
                            start=(t + tl == 0),
                            stop=(t + tl == t_per_core - 1),
                        )
                t += bsz
            outt = op.tile([128, 256], mybir.dt.float32, tag="outt")
            for g in range(2):
                nc.vector.tensor_copy(
                    out=outt[:, g * 128 : (g + 1) * 128], in_=ps[g]
                )
            nc.sync.dma_start(out=gram[:], in_=outt[:])

    nc.compile()
    _nc_cache[key] = nc
    return nc


def build_nc_raw(t_per_core=T_PER_CORE, blocks=None, warmup_mms=128, credit_window=7):
    """Raw bacc kernel: the whole per-core working set (16 MiB bf16) fits in
    SBUF, so all input DMAs are emitted upfront with no PE-gated credits —
    the stream runs at full HBM rate end to end. PE pre-warms its clock gate
    during the first DMA, then consumes blocks as they land."""
    if blocks is None:
        if t_per_core == T_PER_CORE:
            # small blocks first (fast pipeline fill), big in the middle
            # (descriptor efficiency), small at the end (short tail)
            blocks = [4, 4, 8, 16] + [32] * 6 + [16, 8, 4, 4]
        else:
            blocks = [t_per_core // 2] * 2
    assert sum(blocks) == t_per_core
    key = ("raw", t_per_core, tuple(blocks), warmup_mms, credit_window)
    if key in _nc_cache:
        return _nc_cache[key]

    nblocks = len(blocks)
    f32 = mybir.dt.float32
    bf16 = mybir.dt.bfloat16

    nc = bacc.Bacc(None, target_bir_lowering=False, debug=False)
    xt = nc.dram_tensor("xt", [128, t_per_core, ROWS], f32, kind="ExternalInput")
    gram = nc.dram_tensor("gram", [128, 256], f32, kind="ExternalOutput")

    # block start offsets
    starts = []
    t = 0
    for b in blocks:
        starts.append(t)
        t += b

    with (
        nc.sbuf_tensor([128, t_per_core, ROWS], bf16) as xbuf,
        nc.sbuf_tensor([128, 128], bf16) as warm_buf,
        nc.sbuf_tensor([128, 256], f32) as outt,
        nc.psum_tensor([128, 128], f32) as ps0,
        nc.psum_tensor([128, 128], f32) as ps1,
        nc.psum_tensor([128, 128], f32) as ps_warm,
        nc.semaphore("warm_sem") as warm_sem,
        nc.semaphore("mm_sem") as mm_sem,
        nc.semaphore("cp_sem") as cp_sem,
        nc.semaphore("out_sem") as out_sem,
    ):
        with ExitStack() as sems_ctx:
            bsems = [
                sems_ctx.enter_context(nc.semaphore(f"bsem{i}"))
                for i in range(nblocks)
            ]

            with nc.Block() as block:

                @block.gpsimd
                def _(g):
                    for i, bsz in enumerate(blocks):
                        if i == 1:
                            # off the critical path: first DMA already going
                            g.memset(warm_buf[:], 0.0).then_inc(warm_sem, 1)
                        # loose credit: bounds SDMA engine skew (the queue
                        # never runs more than ~credit_window blocks ahead
                        # of fully-consumed data) without gating the stream
                        if i >= credit_window:
                            g.wait_ge(mm_sem, i - credit_window + 1)
                        g.dma_start(
                            out=xbuf[:, starts[i] : starts[i] + bsz, :],
                            in_=xt[:, starts[i] : starts[i] + bsz, :],
                        ).then_inc(bsems[i], 16)

                @block.tensor
                def _(te):
                    # pre-warm the PE HAM clock gate while the first DMAs are
                    # in flight (reads a scratch buffer; result goes to a
                    # scratch PSUM bank that is never read)
                    te.wait_ge(warm_sem, 1)
                    for _ in range(warmup_mms):
                        nc.tensor.matmul(
                            ps_warm[:], warm_buf[:], warm_buf[:], start=True, stop=True
                        )
                    for i, bsz in enumerate(blocks):
                        te.wait_ge(bsems[i], 16)
                        last = None
                        for tl in range(bsz):
                            tcur = starts[i] + tl
                            for ps, goff in ((ps0, 0), (ps1, 128)):
                                sl = xbuf[:, tcur, goff : goff + 128]
                                last = nc.tensor.matmul(
                                    ps[:],
                                    sl,
                                    sl,
                                    start=(tcur == 0),
                                    stop=(tcur == t_per_core - 1),
                                )
                        last.then_inc(mm_sem, 1)

                @block.vector
                def _(v):
                    v.wait_ge(mm_sem, nblocks)
                    nc.vector.tensor_copy(out=outt[:, 0:128], in_=ps0[:]).then_inc(
                        cp_sem, 1
                    )

                @block.scalar
                def _(sc):
                    sc.wait_ge(mm_sem, nblocks)
                    nc.scalar.copy(out=outt[:, 128:256], in_=ps1[:]).then_inc(
                        cp_sem, 1
                    )

                @block.sync
                def _(s):
                    s.wait_ge(cp_sem, 2)
                    s.dma_start(out=gram[:], in_=outt[:]).then_inc(out_sem, 16)
                    s.wait_ge(out_sem, 16)

    nc.compile()
    _nc_cache[key] = nc
    return nc


def build_nc_v2(
    t_per_core=T_PER_CORE,
    head_chunks=4,
    head_per_dma=2,
    credit_window=7,
    end_wait=True,
):
    """v2: SP (HWDGE) prefetches the first `head_chunks` t-chunks as raw fp32
    while gpsimd's SWDGE cast-stream is still spinning up (~2.5us of otherwise
    idle HBM time); the PE consumes them as float32r matmuls (full rate at
    moving-dim 256) accumulating into the same PSUM banks the bf16 stream
    uses. Tail blocks shrink to 2 chunks and the PSUM->SBUF copies/output DMA
    run on DVE+Act / SP with minimal chaining."""
    key = ("v2", t_per_core, head_chunks, head_per_dma, credit_window, end_wait)
    if key in _nc_cache:
        return _nc_cache[key]

    # main (gpsimd, bf16-cast) stream covers chunks [head_chunks, t_per_core)
    rest = t_per_core - head_chunks
    taper_in = [4, 8, 16]
    taper_out = [16, 8, 4, 2, 2]
    mid = rest - sum(taper_in) - sum(taper_out)
    assert mid >= 0 and mid % 32 == 0, (rest, mid)
    blocks = taper_in + [32] * (mid // 32) + taper_out
    assert sum(blocks) == rest
    nblocks = len(blocks)
    starts = []
    t = head_chunks
    for b in blocks:
        starts.append(t)
        t += b

    n_head_dmas = head_chunks // head_per_dma
    assert n_head_dmas * head_per_dma == head_chunks

    f32 = mybir.dt.float32
    f32r = mybir.dt.float32r
    bf16 = mybir.dt.bfloat16

    nc = bacc.Bacc(None, target_bir_lowering=False, debug=False)
    xt = nc.dram_tensor("xt", [128, t_per_core, ROWS], f32, kind="ExternalInput")
    gram = nc.dram_tensor("gram", [128, 256], f32, kind="ExternalOutput")

    with (
        nc.sbuf_tensor([128, t_per_core, ROWS], bf16) as xbuf,
        nc.sbuf_tensor([128, max(head_chunks, 1), ROWS], f32) as hstage,
        nc.sbuf_tensor([128, 256], f32) as outt,
        nc.psum_tensor([128, 256], f32) as ps0,
        nc.psum_tensor([128, 256], f32) as ps1,
        nc.semaphore("mm_sem") as mm_sem,
        nc.semaphore("cp_sem") as cp_sem,
        nc.semaphore("out_sem") as out_sem,
    ):
        with ExitStack() as sems_ctx:
            hsems = [
                sems_ctx.enter_context(nc.semaphore(f"hsem{i}"))
                for i in range(max(n_head_dmas, 1))
            ]
            bsems = [
                sems_ctx.enter_context(nc.semaphore(f"bsem{i}"))
                for i in range(nblocks)
            ]

            with nc.Block() as block:

                @block.sync
                def _(s):
                    # head prefetch: raw fp32, lands while gpsimd's SWDGE
                    # pipeline is still starting up
                    for h in range(n_head_dmas):
                        lo = h * head_per_dma
                        s.dma_start(
                            out=hstage[:, lo : lo + head_per_dma, :],
                            in_=xt[:, lo : lo + head_per_dma, :],
                        ).then_inc(hsems[h], 16)
                    # output: single [128,256] fp32 DMA once both copies land
                    s.wait_ge(cp_sem, 2)
                    d = s.dma_start(out=gram[:], in_=outt[:])
                    if end_wait:
                        d.then_inc(out_sem, 16)
                        s.wait_ge(out_sem, 16)

                @block.gpsimd
                def _(g):
                    for i, bsz in enumerate(blocks):
                        if i >= credit_window:
                            g.wait_ge(mm_sem, i - credit_window + 1)
                        g.dma_start(
                            out=xbuf[:, starts[i] : starts[i] + bsz, :],
                            in_=xt[:, starts[i] : starts[i] + bsz, :],
                        ).then_inc(bsems[i], 16)

                @block.tensor
                def _(te):
                    # head: fp32 data consumed directly (4 cyc/row is fine --
                    # these matmuls have the whole stream's slack). ps0 holds
                    # D[g0, :], ps1 holds D[g1, :]; the bf16 stream later
                    # accumulates into the diagonal 128-col halves of the
                    # same banks.
                    for h in range(n_head_dmas):
                        te.wait_ge(hsems[h], 16)
                        for tl in range(head_per_dma):
                            tcur = h * head_per_dma + tl
                            mov = hstage[:, tcur, :]
                            for ps, goff in ((ps0, 0), (ps1, 128)):
                                nc.tensor.matmul(
                                    ps[:, :],
                                    hstage[:, tcur, goff : goff + 128],
                                    mov,
                                    start=(tcur == 0),
                                    stop=False,
                                    skip_group_check=True,
                                )
                    for i, bsz in enumerate(blocks):
                        te.wait_ge(bsems[i], 16)
                        last = None
                        for tl in range(bsz):
                            tcur = starts[i] + tl
                            for ps, goff in ((ps0, 0), (ps1, 128)):
                                sl = xbuf[:, tcur, goff : goff + 128]
                                last = nc.tensor.matmul(
                                    ps[:, goff : goff + 128],
                                    sl,
                                    sl,
                                    start=False,
                                    stop=(tcur == t_per_core - 1),
                                    skip_group_check=True,
                                )
                        last.then_inc(mm_sem, 1)

                @block.vector
                def _(v):
                    v.wait_ge(mm_sem, nblocks)
                    nc.vector.tensor_copy(
                        out=outt[:, 0:128], in_=ps0[:, 0:128]
                    ).then_inc(cp_sem, 1)

                @block.scalar
                def _(sc):
                    sc.wait_ge(mm_sem, nblocks)
                    nc.scalar.copy(
                        out=outt[:, 128:256], in_=ps1[:, 128:256]
                    ).then_inc(cp_sem, 1)

    nc.compile()
    _nc_cache[key] = nc
    return nc


def build_nc_v3(t_per_core=T_PER_CORE, nslots=8, end_wait=True):
    """v3: no SWDGE at all. Both HWDGE queues (SP + Act) stream the fp32
    input into an SBUF ring; the PE consumes it directly as float32r
    matmuls (moving dim 256). Kills the SWDGE descriptor-ring fetch burden
    that made one DMA engine the stream straggler, and starts the stream
    ~1.5us earlier (HWDGE gen at SP main-start)."""
    key = ("v3", t_per_core, nslots, end_wait)
    if key in _nc_cache:
        return _nc_cache[key]

    blocks = [16] * ((t_per_core - 16) // 16) + [8, 4, 2, 1, 1]
    assert sum(blocks) == t_per_core
    nblocks = len(blocks)
    slot_chunks = max(blocks)
    starts = []
    t = 0
    for b in blocks:
        starts.append(t)
        t += b
    # SP issues blocks 0,1 (Act pays its table-load preamble first), then
    # they alternate, including through the taper, so the queues stay
    # byte-balanced (SP 133 / Act 123 chunks) and both drain to tiny final
    # DMAs together.
    owner = [
        "sp" if (i < 2 or (i % 2 == 1 and i <= 13) or i in (16, 18)) else "act"
        for i in range(nblocks)
    ]

    f32 = mybir.dt.float32
    f32r = mybir.dt.float32r

    nc = bacc.Bacc(None, target_bir_lowering=False, debug=False)
    xt = nc.dram_tensor("xt", [128, t_per_core, ROWS], f32r, kind="ExternalInput")
    gram = nc.dram_tensor("gram", [128, 256], f32, kind="ExternalOutput")

    with (
        nc.sbuf_tensor([128, nslots, slot_chunks, ROWS], f32r) as ring,
        nc.sbuf_tensor([128, 256], f32) as outt,
        nc.psum_tensor([128, 256], f32) as ps0,
        nc.psum_tensor([128, 256], f32) as ps1,
        nc.semaphore("pe_sem") as pe_sem,
        nc.semaphore("cp_sem") as cp_sem,
        nc.semaphore("out_sem") as out_sem,
    ):
        with ExitStack() as sems_ctx:
            bsems = [
                sems_ctx.enter_context(nc.semaphore(f"bsem{i}"))
                for i in range(nblocks)
            ]

            def issue_stream(q, who):
                for i, bsz in enumerate(blocks):
                    if owner[i] != who:
                        continue
                    if i >= nslots:
                        # slot free once PE consumed the block that last
                        # used it
                        q.wait_ge(pe_sem, i - nslots + 1)
                    q.dma_start(
                        out=ring[:, i % nslots, :bsz, :],
                        in_=xt[:, starts[i] : starts[i] + bsz, :],
                    ).then_inc(bsems[i], 16)

            with nc.Block() as block:

                @block.sync
                def _(s):
                    issue_stream(s, "sp")
                    s.wait_ge(cp_sem, 2)
                    d = s.dma_start(out=gram[:], in_=outt[:])
                    if end_wait:
                        d.then_inc(out_sem, 16)
                        s.wait_ge(out_sem, 16)

                @block.scalar
                def _(sc):
                    issue_stream(sc, "act")
                    sc.wait_ge(pe_sem, nblocks)
                    nc.scalar.copy(
                        out=outt[:, 128:256], in_=ps1[:, 128:256]
                    ).then_inc(cp_sem, 1)

                @block.tensor
                def _(te):
                    for i, bsz in enumerate(blocks):
                        te.wait_ge(bsems[i], 16)
                        last = None
                        for tl in range(bsz):
                            tcur = starts[i] + tl
                            mov = ring[:, i % nslots, tl, :]
                            for ps, goff in ((ps0, 0), (ps1, 128)):
                                last = nc.tensor.matmul(
                                    ps[:, :],
                                    ring[
                                        :, i % nslots, tl, goff : goff + 128
                                    ],
                                    mov,
                                    start=(tcur == 0),
                                    stop=(tcur == t_per_core - 1),
                                    skip_group_check=True,
                                )
                        last.then_inc(pe_sem, 1)

                @block.vector
                def _(v):
                    v.wait_ge(pe_sem, nblocks)
                    nc.vector.tensor_copy(
                        out=outt[:, 0:128], in_=ps0[:, 0:128]
                    ).then_inc(cp_sem, 1)

    nc.compile()
    _nc_cache[key] = nc
    return nc


def build_nc_v4(t_per_core=T_PER_CORE, nslots=5, end_wait=True):
    """v4 = v3 with a fixed schedule: taper-in so the PE starts ~8.7us (not
    18.5), byte-balanced SP/Act queues in strict alternation so blocks
    complete in consumption order, and 1-chunk final blocks on both queues
    for a minimal tail."""
    key = ("v4", t_per_core, nslots, end_wait)
    if key in _nc_cache:
        return _nc_cache[key]

    sizes = [2, 2, 4, 8, 16] + [32] * 6 + [16, 8, 4, 2, 1, 1]
    assert sum(sizes) == t_per_core
    nblocks = len(sizes)
    slot_chunks = max(sizes)
    starts = []
    t = 0
    for b in sizes:
        starts.append(t)
        t += b
    # SP opens (Act pays its table-load preamble), then strict alternation;
    # bytes balance to 129/127 chunks and both queues end on a 1-chunk DMA.
    owner = ["sp" if (i < 2 or i % 2 == 1) else "act" for i in range(nblocks)]

    f32 = mybir.dt.float32
    f32r = mybir.dt.float32r

    nc = bacc.Bacc(None, target_bir_lowering=False, debug=False)
    xt = nc.dram_tensor("xt", [128, t_per_core, ROWS], f32r, kind="ExternalInput")
    gram = nc.dram_tensor("gram", [128, 256], f32, kind="ExternalOutput")

    with (
        nc.sbuf_tensor([128, nslots, slot_chunks, ROWS], f32r) as ring,
        nc.sbuf_tensor([128, 256], f32) as outt,
        nc.psum_tensor([128, 256], f32) as ps0,
        nc.psum_tensor([128, 256], f32) as ps1,
        nc.semaphore("pe_sem") as pe_sem,
        nc.semaphore("cp_sem") as cp_sem,
        nc.semaphore("out_sem") as out_sem,
    ):
        with ExitStack() as sems_ctx:
            bsems = [
                sems_ctx.enter_context(nc.semaphore(f"bsem{i}"))
                for i in range(nblocks)
            ]

            def issue_stream(q, who):
                for i, bsz in enumerate(sizes):
                    if owner[i] != who:
                        continue
                    if i >= nslots:
                        q.wait_ge(pe_sem, i - nslots + 1)
                    q.dma_start(
                        out=ring[:, i % nslots, :bsz, :],
                        in_=xt[:, starts[i] : starts[i] + bsz, :],
                    ).then_inc(bsems[i], 16)

            with nc.Block() as block:

                @block.sync
                def _(s):
                    issue_stream(s, "sp")
                    s.wait_ge(cp_sem, 2)
                    d = s.dma_start(out=gram[:], in_=outt[:])
                    if end_wait:
                        d.then_inc(out_sem, 16)
                        s.wait_ge(out_sem, 16)

                @block.scalar
                def _(sc):
                    issue_stream(sc, "act")
                    sc.wait_ge(pe_sem, nblocks)
                    nc.scalar.copy(
                        out=outt[:, 128:256], in_=ps1[:, 128:256]
                    ).then_inc(cp_sem, 1)

                @block.tensor
                def _(te):
                    for i, bsz in enumerate(sizes):
                        te.wait_ge(bsems[i], 16)
                        last = None
                        for tl in range(bsz):
                            tcur = starts[i] + tl
                            mov = ring[:, i % nslots, tl, :]
                            for ps, goff in ((ps0, 0), (ps1, 128)):
                                last = nc.tensor.matmul(
                                    ps[:, :],
                                    ring[
                                        :, i % nslots, tl, goff : goff + 128
                                    ],
                                    mov,
                                    start=(tcur == 0),
                                    stop=(tcur == t_per_core - 1),
                                    skip_group_check=True,
                                )
                        last.then_inc(pe_sem, 1)

                @block.vector
                def _(v):
                    v.wait_ge(pe_sem, nblocks)
                    nc.vector.tensor_copy(
                        out=outt[:, 0:128], in_=ps0[:, 0:128]
                    ).then_inc(cp_sem, 1)

    nc.compile()
    _nc_cache[key] = nc
    return nc


def build_nc_v5(t_per_core=T_PER_CORE, ring_chunks=160, end_wait=True):
    """v5 = v4 with a flat chunk-position ring (no per-block slots): block i
    lives at ring chunk starts[i] % ring_chunks, sized so no block wraps.
    Credits are computed from real chunk distances, so with a 160-chunk ring
    every taper DMA is issued far ahead of need and the stream has no
    end-of-kernel serialization."""
    key = ("v5", t_per_core, ring_chunks, end_wait)
    if key in _nc_cache:
        return _nc_cache[key]

    sizes = [2, 2, 4, 8, 16] + [32] * 6 + [16, 8, 4, 2, 1, 1]
    assert sum(sizes) == t_per_core
    nblocks = len(sizes)
    starts = []
    t = 0
    for b in sizes:
        starts.append(t)
        t += b
    for i, b in enumerate(sizes):
        assert starts[i] % ring_chunks + b <= ring_chunks, (i, starts[i], b)
    owner = ["sp" if (i < 2 or i % 2 == 1) else "act" for i in range(nblocks)]

    # credit threshold: block i may load once PE has consumed through chunk
    # starts[i] + size - ring_chunks, i.e. pe_sem >= (number of whole blocks
    # covering those chunks)
    def credit(i):
        need = starts[i] + sizes[i] - ring_chunks
        if need <= 0:
            return 0
        j = 0
        while starts[j] < need:
            j += 1
        return j  # pe_sem counts fully-consumed blocks

    f32 = mybir.dt.float32
    f32r = mybir.dt.float32r

    nc = bacc.Bacc(None, target_bir_lowering=False, debug=False)
    xt = nc.dram_tensor("xt", [128, t_per_core, ROWS], f32r, kind="ExternalInput")
    gram = nc.dram_tensor("gram", [128, 256], f32, kind="ExternalOutput")

    with (
        nc.sbuf_tensor([128, ring_chunks, ROWS], f32r) as ring,
        nc.sbuf_tensor([128, 256], f32) as outt,
        nc.psum_tensor([128, 256], f32) as ps0,
        nc.psum_tensor([128, 256], f32) as ps1,
        nc.semaphore("pe_sem") as pe_sem,
        nc.semaphore("cp_sem") as cp_sem,
        nc.semaphore("out_sem") as out_sem,
    ):
        with ExitStack() as sems_ctx:
            bsems = [
                sems_ctx.enter_context(nc.semaphore(f"bsem{i}"))
                for i in range(nblocks)
            ]

            def issue_stream(q, who):
                for i, bsz in enumerate(sizes):
                    if owner[i] != who:
                        continue
                    c = credit(i)
                    if c > 0:
                        q.wait_ge(pe_sem, c)
                    pos = starts[i] % ring_chunks
                    q.dma_start(
                        out=ring[:, pos : pos + bsz, :],
                        in_=xt[:, starts[i] : starts[i] + bsz, :],
                    ).then_inc(bsems[i], 16)

            with nc.Block() as block:

                @block.sync
                def _(s):
                    issue_stream(s, "sp")
                    s.wait_ge(cp_sem, 2)
                    d = s.dma_start(out=gram[:], in_=outt[:])
                    if end_wait:
                        d.then_inc(out_sem, 16)
                        s.wait_ge(out_sem, 16)

                @block.scalar
                def _(sc):
                    issue_stream(sc, "act")
                    sc.wait_ge(pe_sem, nblocks)
                    nc.scalar.copy(
                        out=outt[:, 128:256], in_=ps1[:, 128:256]
                    ).then_inc(cp_sem, 1)

                @block.tensor
                def _(te):
                    for i, bsz in enumerate(sizes):
                        te.wait_ge(bsems[i], 16)
                        last = None
                        for tl in range(bsz):
                            tcur = starts[i] + tl
                            pos = starts[i] % ring_chunks + tl
                            mov = ring[:, pos, :]
                            for ps, goff in ((ps0, 0), (ps1, 128)):
                                last = nc.tensor.matmul(
                                    ps[:, :],
                                    ring[:, pos, goff : goff + 128],
                                    mov,
                                    start=(tcur == 0),
                                    stop=(tcur == t_per_core - 1),
                                    skip_group_check=True,
                                )
                        last.then_inc(pe_sem, 1)

                @block.vector
                def _(v):
                    v.wait_ge(pe_sem, nblocks)
                    nc.vector.tensor_copy(
                        out=outt[:, 0:128], in_=ps0[:, 0:128]
                    ).then_inc(cp_sem, 1)

    nc.compile()
    _nc_cache[key] = nc
    return nc


def build_nc_hwdge(t_per_core=T_PER_CORE, warmup_mms=128, nstage=3):
    """HWDGE loads (immune to the SWDGE descriptor-ring engine-7/15
    contention): fp32 staged via a 3-slot ring, cast to bf16 on DVE into the
    resident xbuf, PE consumes per block. Same math as build_nc_raw."""
    if t_per_core == T_PER_CORE:
        blocks = [16] * 15 + [8, 4, 4]
    else:
        blocks = [t_per_core // 2] * 2
    assert sum(blocks) == t_per_core
    key = ("hwdge", t_per_core, warmup_mms, nstage)
    if key in _nc_cache:
        return _nc_cache[key]

    nblocks = len(blocks)
    max_b = max(blocks)
    f32 = mybir.dt.float32
    bf16 = mybir.dt.bfloat16

    nc = bacc.Bacc(None, target_bir_lowering=False, debug=False)
    xt = nc.dram_tensor("xt", [128, t_per_core, ROWS], f32, kind="ExternalInput")
    gram = nc.dram_tensor("gram", [128, 256], f32, kind="ExternalOutput")

    starts = []
    t = 0
    for b in blocks:
        starts.append(t)
        t += b

    with (
        nc.sbuf_tensor([128, t_per_core, ROWS], bf16) as xbuf,
        nc.sbuf_tensor([128, nstage, max_b, ROWS], f32) as stage,
        nc.sbuf_tensor([128, 128], bf16) as warm_buf,
        nc.sbuf_tensor([128, 256], f32) as outt,
        nc.psum_tensor([128, 128], f32) as ps0,
        nc.psum_tensor([128, 128], f32) as ps1,
        nc.psum_tensor([128, 128], f32) as ps_warm,
        nc.semaphore("warm_sem") as warm_sem,
        nc.semaphore("cast_done") as cast_done,
        nc.semaphore("mm_sem") as mm_sem,
        nc.semaphore("cp_sem") as cp_sem,
        nc.semaphore("out_sem") as out_sem,
    ):
        with ExitStack() as sems_ctx:
            ssems = [
                sems_ctx.enter_context(nc.semaphore(f"ssem{s}"))
                for s in range(nstage)
            ]

            with nc.Block() as block:

                @block.gpsimd
                def _(g):
                    g.memset(warm_buf[:], 0.0).then_inc(warm_sem, 1)

                @block.sync
                def _(s):
                    for i, bsz in enumerate(blocks):
                        if i >= nstage:
                            # slot free once its previous block is cast
                            s.wait_ge(cast_done, i - nstage + 1)
                        s.dma_start(
                            out=stage[:, i % nstage, :bsz, :],
                            in_=xt[:, starts[i] : starts[i] + bsz, :],
                        ).then_inc(ssems[i % nstage], 16)
                    # output: wait for both PSUM copies, DMA out, drain
                    s.wait_ge(cp_sem, 2)
                    s.dma_start(out=gram[:], in_=outt[:]).then_inc(out_sem, 16)
                    s.wait_ge(out_sem, 16)

                @block.vector
                def _(v):
                    for i, bsz in enumerate(blocks):
                        v.wait_ge(ssems[i % nstage], 16 * (i // nstage + 1))
                        nc.vector.tensor_copy(
                            out=xbuf[:, starts[i] : starts[i] + bsz, :],
                            in_=stage[:, i % nstage, :bsz, :],
                        ).then_inc(cast_done, 1)

                @block.tensor
                def _(te):
                    te.wait_ge(warm_sem, 1)
                    for _ in range(warmup_mms):
                        nc.tensor.matmul(
                            ps_warm[:], warm_buf[:], warm_buf[:], start=True, stop=True
                        )
                    for i, bsz in enumerate(blocks):
                        te.wait_ge(cast_done, i + 1)
                        last = None
                        for tl in range(bsz):
                            tcur = starts[i] + tl
                            for ps, goff in ((ps0, 0), (ps1, 128)):
                                sl = xbuf[:, tcur, goff : goff + 128]
                                last = nc.tensor.matmul(
                                    ps[:],
                                    sl,
                                    sl,
                                    start=(tcur == 0),
                                    stop=(tcur == t_per_core - 1),
                                )
                        if i == nblocks - 1:
                            last.then_inc(mm_sem, 1)

                @block.scalar
                def _(sc):
                    sc.wait_ge(mm_sem, 1)
                    nc.scalar.copy(out=outt[:, 0:128], in_=ps0[:]).then_inc(cp_sem, 1)
                    nc.scalar.copy(out=outt[:, 128:256], in_=ps1[:]).then_inc(
                        cp_sem, 1
                    )

    nc.compile()
    _nc_cache[key] = nc
    return nc


def shard_inputs(pred):
    """[32, 8, 512, 512] fp32 -> per-core [128, T_PER_CORE, 256] arrays.

    Per-core layout: xt[p, t, m] = x[m, c*32768 + t*128 + p] where
    x = pred.reshape(256, 262144). Done in cache-friendly stages.
    """
    x = np.ascontiguousarray(pred, dtype=np.float32).reshape(ROWS, L // 128, 128)
    # stage 1: [m, T, p] -> [T, m, p]   (inner 512B runs are contiguous)
    g = np.ascontiguousarray(x.transpose(1, 0, 2))
    # stage 2: [T, m, p] -> [T, p, m]   (per-T 128 KiB slice, cache resident)
    h = np.ascontiguousarray(g.transpose(0, 2, 1))
    # stage 3: [c*t, p, m] -> [c, p, t, m]  (inner 1 KiB contiguous runs)
    xt = np.ascontiguousarray(
        h.reshape(N_CORES, T_PER_CORE, 128, ROWS).transpose(0, 2, 1, 3)
    )
    return xt


def postprocess(gram_list):
    """Sum per-core partial Grams and reduce to the scalar loss."""
    d = np.zeros((128, 256), dtype=np.float64)
    for garr in gram_list:
        d += np.asarray(garr, dtype=np.float64)
    total = 0.0
    for b in range(B):
        g, j = divmod(b, 16)
        blk = d[8 * j : 8 * j + 8, g * 128 + 8 * j : g * 128 + 8 * j + 8]
        norms = np.sqrt(np.maximum(np.diag(blk), 0.0))
        denom = np.maximum(norms, EPS)
        gn = blk / np.outer(denom, denom)
        np.fill_diagonal(gn, 1.0)
        total += gn.sum()
    return np.asarray(total / (B * NMAP * NMAP), dtype=np.float32)


KERNEL_MODE = os.environ.get("KERNEL_MODE", "raw")


def run(pred, trace=False, **spmd_kwargs):
    pred = np.asarray(pred, dtype=np.float32)
    assert pred.shape == (B, NMAP, H, W), pred.shape
    if KERNEL_MODE == "raw":
        nc = build_nc_raw()
    elif KERNEL_MODE == "hwdge":
        nc = build_nc_hwdge()
    elif KERNEL_MODE == "v2":
        nc = build_nc_v2(end_wait=os.environ.get("END_WAIT", "1") == "1")
    elif KERNEL_MODE == "v3":
        nc = build_nc_v3(end_wait=os.environ.get("END_WAIT", "1") == "1")
    elif KERNEL_MODE == "v4":
        nc = build_nc_v4(end_wait=os.environ.get("END_WAIT", "1") == "1")
    elif KERNEL_MODE == "v5":
        nc = build_nc_v5(end_wait=os.environ.get("END_WAIT", "1") == "1")
    else:
        nc = build_nc()
    xt = shard_inputs(pred)
    in_maps = [{"xt": xt[c]} for c in range(N_CORES)]
    res = run_bass_kernel_spmd(
        nc, in_maps, core_ids=list(range(N_CORES)), trace=trace, **spmd_kwargs
    )
    value = postprocess([r["gram"] for r in res.results])
    return value, res


def kernel(pred):
    value, _ = run(pred, trace=False)
    return value



# revision 23
# speedup vs baseline: 1.2996x; 1.0187x over previous
"""Trainium2 Bass kernel for nn_CosSim_Loss.

Computes mean of per-batch cosine-similarity Gram matrices of
pred [32, 8, 512, 512] -> scalar.

Strategy: shard the contraction dim L = 512*512 = 262144 across the 8
cores (each core gets L/8 = 32768 contiguous elements of every row).
Each core computes the partial Gram sums D[m, n] = sum_l x[m, l] x[n, l]
for the two 128-row groups (rows = 32 batches x 8 maps = 256) with
TensorE matmuls (contraction on partitions), accumulating in PSUM over
256 k-chunks. The host sums the 8 per-core partial Grams, extracts the
per-batch 8x8 diagonal blocks, normalizes by the row norms (taken from
the Gram diagonal) and takes the mean, with the diagonal forced to
exactly 1.0 like the reference.

The data is fed to each core pre-transposed ([p, t, m] with l-chunk on
partitions) so the device DMAs are dense 16 KiB+/partition descriptors
and no on-chip transpose is needed; the hardware reads the full
256 MiB of fp32 input (memory-roofline regime, ~400 GB/s/core).

Default mode "v2" (build_nc_v2): the main stream is gpsimd/SWDGE DMAs
that cast fp32->bf16 inline into a resident SBUF buffer, consumed by
bf16 matmuls. Its first descriptor generation can't start until the
gpsimd sequencer boots (~7.4 us into the program), so the first ~2.5 us
of HBM time would be idle; instead the SP (sync) HWDGE queue prefetches
the first 4 k-chunks as raw fp32 at main-start and the PE consumes
those as fp32 matmuls (they have the whole stream's slack), accumulating
into the same PSUM banks. The tail skips the final out_sem wait (the
teardown drain covers the output DMA), saving the ~900 ns DMA-semaphore
propagation plus wait dispatch.

Measured (core-0 NEFF exec, good mode): ~94.6-95.8 us vs ~97.6-99.3 us
for the previous best (KERNEL_MODE=raw). Note the device shows a
bimodal ~+13 us "hot DMA engine" mode on ~40% of runs (one engine's
descriptor slices stretch 10-30x; independent of kernel structure).

Alternative modes kept for reference/experiments: raw (previous best),
v3/v4/v5 (all-HWDGE fp32r streaming - correct, ~7 us slower in good
mode because the PE fp32r rate of ~240 ns/chunk barely exceeds the
~328 ns/chunk arrival rate and queue skew turns into PE backlog),
hwdge (staged fp32 + DVE cast).
"""

import os
import sys
from contextlib import ExitStack

import numpy as np

for _p in ("/opt/trn_rl_repo", "/root/.axon_site/_ro/trn_rl_repo"):
    if os.path.isdir(_p) and _p not in sys.path:
        sys.path.append(_p)

import concourse.bass as bass  # noqa: E402
import concourse.mybir as mybir  # noqa: E402
from concourse import bacc  # noqa: E402
from concourse.bass_utils import run_bass_kernel_spmd  # noqa: E402
from concourse.tile import TileContext  # noqa: E402

N_CORES = 8
B, NMAP, H, W = 32, 8, 512, 512
L = H * W  # 262144
ROWS = B * NMAP  # 256
L_SHARD = L // N_CORES  # 32768
T_PER_CORE = L_SHARD // 128  # 256
EPS = 1e-8
NBLK = 16  # t-chunks per DMA (2 MiB fp32 read -> 1 MiB bf16 in SBUF)

_nc_cache = {}


def build_nc(t_per_core=T_PER_CORE, nblk=NBLK):
    """Build + compile the per-core Bass program (same program on all cores)."""
    key = (t_per_core, nblk)
    if key in _nc_cache:
        return _nc_cache[key]

    nc = bacc.Bacc(None, target_bir_lowering=False, debug=False)
    xt = nc.dram_tensor(
        "xt", [128, t_per_core, ROWS], mybir.dt.float32, kind="ExternalInput"
    )
    gram = nc.dram_tensor("gram", [128, 256], mybir.dt.float32, kind="ExternalOutput")

    # block sizes (t-chunks per DMA): big blocks stream at full HBM rate
    blocks = [nblk] * (t_per_core // nblk)
    assert sum(blocks) == t_per_core

    with TileContext(nc) as tc:
        with (
            tc.tile_pool(name="load", bufs=6) as lp,
            tc.tile_pool(name="psum", bufs=1, space=bass.MemorySpace.PSUM) as pp,
            tc.tile_pool(name="outp", bufs=1) as op,
        ):
            ps = [
                pp.tile([128, 128], mybir.dt.float32, name=f"ps{g}", tag=f"ps{g}")
                for g in range(2)
            ]
            t = 0
            max_b = max(blocks)
            for bsz in blocks:
                bt = lp.tile([128, max_b, ROWS], mybir.dt.bfloat16, tag="bt")
                # gpsimd (SWDGE) DMA casts fp32 -> bf16 inline
                nc.gpsimd.dma_start(
                    out=bt[:, :bsz, :], in_=xt[:, t : t + bsz, :]
                )
                for tl in range(bsz):
                    for g in range(2):
                        sl = bt[:, tl, g * 128 : (g + 1) * 128]
                        nc.tensor.matmul(
                            ps[g],
                            sl,
                            sl,
                            start=(t + tl == 0),
                            stop=(t + tl == t_per_core - 1),
                        )
                t += bsz
            outt = op.tile([128, 256], mybir.dt.float32, tag="outt")
            for g in range(2):
                nc.vector.tensor_copy(
                    out=outt[:, g * 128 : (g + 1) * 128], in_=ps[g]
                )
            nc.sync.dma_start(out=gram[:], in_=outt[:])

    nc.compile()
    _nc_cache[key] = nc
    return nc


def build_nc_raw(t_per_core=T_PER_CORE, blocks=None, warmup_mms=128, credit_window=7):
    """Raw bacc kernel: the whole per-core working set (16 MiB bf16) fits in
    SBUF, so all input DMAs are emitted upfront with no PE-gated credits —
    the stream runs at full HBM rate end to end. PE pre-warms its clock gate
    during the first DMA, then consumes blocks as they land."""
    if blocks is None:
        if t_per_core == T_PER_CORE:
            # small blocks first (fast pipeline fill), big in the middle
            # (descriptor efficiency), small at the end (short tail)
            blocks = [4, 4, 8, 16] + [32] * 6 + [16, 8, 4, 4]
        else:
            blocks = [t_per_core // 2] * 2
    assert sum(blocks) == t_per_core
    key = ("raw", t_per_core, tuple(blocks), warmup_mms, credit_window)
    if key in _nc_cache:
        return _nc_cache[key]

    nblocks = len(blocks)
    f32 = mybir.dt.float32
    bf16 = mybir.dt.bfloat16

    nc = bacc.Bacc(None, target_bir_lowering=False, debug=False)
    xt = nc.dram_tensor("xt", [128, t_per_core, ROWS], f32, kind="ExternalInput")
    gram = nc.dram_tensor("gram", [128, 256], f32, kind="ExternalOutput")

    # block start offsets
    starts = []
    t = 0
    for b in blocks:
        starts.append(t)
        t += b

    with (
        nc.sbuf_tensor([128, t_per_core, ROWS], bf16) as xbuf,
        nc.sbuf_tensor([128, 128], bf16) as warm_buf,
        nc.sbuf_tensor([128, 256], f32) as outt,
        nc.psum_tensor([128, 128], f32) as ps0,
        nc.psum_tensor([128, 128], f32) as ps1,
        nc.psum_tensor([128, 128], f32) as ps_warm,
        nc.semaphore("warm_sem") as warm_sem,
        nc.semaphore("mm_sem") as mm_sem,
        nc.semaphore("cp_sem") as cp_sem,
        nc.semaphore("out_sem") as out_sem,
    ):
        with ExitStack() as sems_ctx:
            bsems = [
                sems_ctx.enter_context(nc.semaphore(f"bsem{i}"))
                for i in range(nblocks)
            ]

            with nc.Block() as block:

                @block.gpsimd
                def _(g):
                    for i, bsz in enumerate(blocks):
                        if i == 1:
                            # off the critical path: first DMA already going
                            g.memset(warm_buf[:], 0.0).then_inc(warm_sem, 1)
                        # loose credit: bounds SDMA engine skew (the queue
                        # never runs more than ~credit_window blocks ahead
                        # of fully-consumed data) without gating the stream
                        if i >= credit_window:
                            g.wait_ge(mm_sem, i - credit_window + 1)
                        g.dma_start(
                            out=xbuf[:, starts[i] : starts[i] + bsz, :],
                            in_=xt[:, starts[i] : starts[i] + bsz, :],
                        ).then_inc(bsems[i], 16)

                @block.tensor
                def _(te):
                    # pre-warm the PE HAM clock gate while the first DMAs are
                    # in flight (reads a scratch buffer; result goes to a
                    # scratch PSUM bank that is never read)
                    te.wait_ge(warm_sem, 1)
                    for _ in range(warmup_mms):
                        nc.tensor.matmul(
                            ps_warm[:], warm_buf[:], warm_buf[:], start=True, stop=True
                        )
                    for i, bsz in enumerate(blocks):
                        te.wait_ge(bsems[i], 16)
                        last = None
                        for tl in range(bsz):
                            tcur = starts[i] + tl
                            for ps, goff in ((ps0, 0), (ps1, 128)):
                                sl = xbuf[:, tcur, goff : goff + 128]
                                last = nc.tensor.matmul(
                                    ps[:],
                                    sl,
                                    sl,
                                    start=(tcur == 0),
                                    stop=(tcur == t_per_core - 1),
                                )
                        last.then_inc(mm_sem, 1)

                @block.vector
                def _(v):
                    v.wait_ge(mm_sem, nblocks)
                    nc.vector.tensor_copy(out=outt[:, 0:128], in_=ps0[:]).then_inc(
                        cp_sem, 1
                    )

                @block.scalar
                def _(sc):
                    sc.wait_ge(mm_sem, nblocks)
                    nc.scalar.copy(out=outt[:, 128:256], in_=ps1[:]).then_inc(
                        cp_sem, 1
                    )

                @block.sync
                def _(s):
                    s.wait_ge(cp_sem, 2)
                    s.dma_start(out=gram[:], in_=outt[:]).then_inc(out_sem, 16)
                    s.wait_ge(out_sem, 16)

    nc.compile()
    _nc_cache[key] = nc
    return nc


def build_nc_v2(
    t_per_core=T_PER_CORE,
    head_chunks=4,
    head_per_dma=2,
    credit_window=7,
    end_wait=True,
    big_block=32,
):
    """v2: SP (HWDGE) prefetches the first `head_chunks` t-chunks as raw fp32
    while gpsimd's SWDGE cast-stream is still spinning up (~2.5us of otherwise
    idle HBM time); the PE consumes them as float32r matmuls (full rate at
    moving-dim 256) accumulating into the same PSUM banks the bf16 stream
    uses. Tail blocks shrink to 2 chunks and the PSUM->SBUF copies/output DMA
    run on DVE+Act / SP with minimal chaining."""
    key = (
        "v2",
        t_per_core,
        head_chunks,
        head_per_dma,
        credit_window,
        end_wait,
        big_block,
    )
    if key in _nc_cache:
        return _nc_cache[key]

    # main (gpsimd, bf16-cast) stream covers chunks [head_chunks, t_per_core)
    rest = t_per_core - head_chunks
    taper_in = [4, 8, 16]
    taper_out = [16, 8, 4, 2, 2]
    mid = rest - sum(taper_in) - sum(taper_out)
    assert mid >= 0 and mid % big_block == 0, (rest, mid)
    blocks = taper_in + [big_block] * (mid // big_block) + taper_out
    assert sum(blocks) == rest
    nblocks = len(blocks)
    starts = []
    t = head_chunks
    for b in blocks:
        starts.append(t)
        t += b

    n_head_dmas = head_chunks // head_per_dma
    assert n_head_dmas * head_per_dma == head_chunks

    f32 = mybir.dt.float32
    f32r = mybir.dt.float32r
    bf16 = mybir.dt.bfloat16

    nc = bacc.Bacc(None, target_bir_lowering=False, debug=False)
    xt = nc.dram_tensor("xt", [128, t_per_core, ROWS], f32, kind="ExternalInput")
    gram = nc.dram_tensor("gram", [128, 256], f32, kind="ExternalOutput")

    with (
        nc.sbuf_tensor([128, t_per_core, ROWS], bf16) as xbuf,
        nc.sbuf_tensor([128, max(head_chunks, 1), ROWS], f32) as hstage,
        nc.sbuf_tensor([128, 256], f32) as outt,
        nc.psum_tensor([128, 256], f32) as ps0,
        nc.psum_tensor([128, 256], f32) as ps1,
        nc.semaphore("mm_sem") as mm_sem,
        nc.semaphore("cp_sem") as cp_sem,
        nc.semaphore("out_sem") as out_sem,
    ):
        with ExitStack() as sems_ctx:
            hsems = [
                sems_ctx.enter_context(nc.semaphore(f"hsem{i}"))
                for i in range(max(n_head_dmas, 1))
            ]
            bsems = [
                sems_ctx.enter_context(nc.semaphore(f"bsem{i}"))
                for i in range(nblocks)
            ]

            with nc.Block() as block:

                @block.sync
                def _(s):
                    # head prefetch: raw fp32, lands while gpsimd's SWDGE
                    # pipeline is still starting up
                    for h in range(n_head_dmas):
                        lo = h * head_per_dma
                        s.dma_start(
                            out=hstage[:, lo : lo + head_per_dma, :],
                            in_=xt[:, lo : lo + head_per_dma, :],
                        ).then_inc(hsems[h], 16)
                    # output: single [128,256] fp32 DMA once both copies land
                    s.wait_ge(cp_sem, 2)
                    s.dma_start(out=gram[:], in_=outt[:]).then_inc(out_sem, 16)
                    if end_wait:
                        s.wait_ge(out_sem, 16)

                @block.gpsimd
                def _(g):
                    for i, bsz in enumerate(blocks):
                        if i >= credit_window:
                            g.wait_ge(mm_sem, i - credit_window + 1)
                        g.dma_start(
                            out=xbuf[:, starts[i] : starts[i] + bsz, :],
                            in_=xt[:, starts[i] : starts[i] + bsz, :],
                        ).then_inc(bsems[i], 16)

                @block.tensor
                def _(te):
                    # head: fp32 data consumed directly (4 cyc/row is fine --
                    # these matmuls have the whole stream's slack). ps0 holds
                    # D[g0, :], ps1 holds D[g1, :]; the bf16 stream later
                    # accumulates into the diagonal 128-col halves of the
                    # same banks.
                    for h in range(n_head_dmas):
                        te.wait_ge(hsems[h], 16)
                        for tl in range(head_per_dma):
                            tcur = h * head_per_dma + tl
                            mov = hstage[:, tcur, :]
                            for ps, goff in ((ps0, 0), (ps1, 128)):
                                nc.tensor.matmul(
                                    ps[:, :],
                                    hstage[:, tcur, goff : goff + 128],
                                    mov,
                                    start=(tcur == 0),
                                    stop=False,
                                    skip_group_check=True,
                                )
                    for i, bsz in enumerate(blocks):
                        te.wait_ge(bsems[i], 16)
                        last = None
                        for tl in range(bsz):
                            tcur = starts[i] + tl
                            for ps, goff in ((ps0, 0), (ps1, 128)):
                                sl = xbuf[:, tcur, goff : goff + 128]
                                last = nc.tensor.matmul(
                                    ps[:, goff : goff + 128],
                                    sl,
                                    sl,
                                    start=False,
                                    stop=(tcur == t_per_core - 1),
                                    skip_group_check=True,
                                )
                        last.then_inc(mm_sem, 1)

                @block.vector
                def _(v):
                    v.wait_ge(mm_sem, nblocks)
                    nc.vector.tensor_copy(
                        out=outt[:, 0:128], in_=ps0[:, 0:128]
                    ).then_inc(cp_sem, 1)

                @block.scalar
                def _(sc):
                    sc.wait_ge(mm_sem, nblocks)
                    nc.scalar.copy(
                        out=outt[:, 128:256], in_=ps1[:, 128:256]
                    ).then_inc(cp_sem, 1)

    nc.compile()
    _nc_cache[key] = nc
    return nc


def build_nc_v3(t_per_core=T_PER_CORE, nslots=8, end_wait=True, owners="bal"):
    """v3: no SWDGE at all. Both HWDGE queues (SP + Act) stream the fp32
    input into an SBUF ring; the PE consumes it directly as float32r
    matmuls (moving dim 256). Kills the SWDGE descriptor-ring fetch burden
    that made one DMA engine the stream straggler, and starts the stream
    ~1.5us earlier (HWDGE gen at SP main-start)."""
    key = ("v3", t_per_core, nslots, end_wait, owners)
    if key in _nc_cache:
        return _nc_cache[key]

    blocks = [16] * ((t_per_core - 16) // 16) + [8, 4, 2, 1, 1]
    assert sum(blocks) == t_per_core
    nblocks = len(blocks)
    slot_chunks = max(blocks)
    starts = []
    t = 0
    for b in blocks:
        starts.append(t)
        t += b
    # SP issues blocks 0,1 (Act pays its table-load preamble first), then
    # they alternate. "bal" balances bytes through the taper (SP 133 /
    # Act 123 chunks, both queues drain to tiny final DMAs together);
    # "orig" is the first v3 assignment (SP 139 / Act 117).
    if owners == "bal":
        owner = [
            "sp"
            if (i < 2 or (i % 2 == 1 and i <= 13) or i in (16, 18))
            else "act"
            for i in range(nblocks)
        ]
    else:
        owner = [
            "sp" if (i < 2 or i % 2 == 1) else "act" for i in range(nblocks)
        ]

    f32 = mybir.dt.float32
    f32r = mybir.dt.float32r

    nc = bacc.Bacc(None, target_bir_lowering=False, debug=False)
    xt = nc.dram_tensor("xt", [128, t_per_core, ROWS], f32r, kind="ExternalInput")
    gram = nc.dram_tensor("gram", [128, 256], f32, kind="ExternalOutput")

    with (
        nc.sbuf_tensor([128, nslots, slot_chunks, ROWS], f32r) as ring,
        nc.sbuf_tensor([128, 256], f32) as outt,
        nc.psum_tensor([128, 256], f32) as ps0,
        nc.psum_tensor([128, 256], f32) as ps1,
        nc.semaphore("pe_sem") as pe_sem,
        nc.semaphore("cp_sem") as cp_sem,
        nc.semaphore("out_sem") as out_sem,
    ):
        with ExitStack() as sems_ctx:
            bsems = [
                sems_ctx.enter_context(nc.semaphore(f"bsem{i}"))
                for i in range(nblocks)
            ]

            def issue_stream(q, who):
                for i, bsz in enumerate(blocks):
                    if owner[i] != who:
                        continue
                    if i >= nslots:
                        # slot free once PE consumed the block that last
                        # used it
                        q.wait_ge(pe_sem, i - nslots + 1)
                    q.dma_start(
                        out=ring[:, i % nslots, :bsz, :],
                        in_=xt[:, starts[i] : starts[i] + bsz, :],
                    ).then_inc(bsems[i], 16)

            with nc.Block() as block:

                @block.sync
                def _(s):
                    issue_stream(s, "sp")
                    s.wait_ge(cp_sem, 2)
                    d = s.dma_start(out=gram[:], in_=outt[:])
                    if end_wait:
                        d.then_inc(out_sem, 16)
                        s.wait_ge(out_sem, 16)

                @block.scalar
                def _(sc):
                    issue_stream(sc, "act")
                    sc.wait_ge(pe_sem, nblocks)
                    nc.scalar.copy(
                        out=outt[:, 128:256], in_=ps1[:, 128:256]
                    ).then_inc(cp_sem, 1)

                @block.tensor
                def _(te):
                    for i, bsz in enumerate(blocks):
                        te.wait_ge(bsems[i], 16)
                        last = None
                        for tl in range(bsz):
                            tcur = starts[i] + tl
                            mov = ring[:, i % nslots, tl, :]
                            for ps, goff in ((ps0, 0), (ps1, 128)):
                                last = nc.tensor.matmul(
                                    ps[:, :],
                                    ring[
                                        :, i % nslots, tl, goff : goff + 128
                                    ],
                                    mov,
                                    start=(tcur == 0),
                                    stop=(tcur == t_per_core - 1),
                                    skip_group_check=True,
                                )
                        last.then_inc(pe_sem, 1)

                @block.vector
                def _(v):
                    v.wait_ge(pe_sem, nblocks)
                    nc.vector.tensor_copy(
                        out=outt[:, 0:128], in_=ps0[:, 0:128]
                    ).then_inc(cp_sem, 1)

    nc.compile()
    _nc_cache[key] = nc
    return nc


def build_nc_v4(t_per_core=T_PER_CORE, nslots=5, end_wait=True):
    """v4 = v3 with a fixed schedule: taper-in so the PE starts ~8.7us (not
    18.5), byte-balanced SP/Act queues in strict alternation so blocks
    complete in consumption order, and 1-chunk final blocks on both queues
    for a minimal tail."""
    key = ("v4", t_per_core, nslots, end_wait)
    if key in _nc_cache:
        return _nc_cache[key]

    sizes = [2, 2, 4, 8, 16] + [32] * 6 + [16, 8, 4, 2, 1, 1]
    assert sum(sizes) == t_per_core
    nblocks = len(sizes)
    slot_chunks = max(sizes)
    starts = []
    t = 0
    for b in sizes:
        starts.append(t)
        t += b
    # SP opens (Act pays its table-load preamble), then strict alternation;
    # bytes balance to 129/127 chunks and both queues end on a 1-chunk DMA.
    owner = ["sp" if (i < 2 or i % 2 == 1) else "act" for i in range(nblocks)]

    f32 = mybir.dt.float32
    f32r = mybir.dt.float32r

    nc = bacc.Bacc(None, target_bir_lowering=False, debug=False)
    xt = nc.dram_tensor("xt", [128, t_per_core, ROWS], f32r, kind="ExternalInput")
    gram = nc.dram_tensor("gram", [128, 256], f32, kind="ExternalOutput")

    with (
        nc.sbuf_tensor([128, nslots, slot_chunks, ROWS], f32r) as ring,
        nc.sbuf_tensor([128, 256], f32) as outt,
        nc.psum_tensor([128, 256], f32) as ps0,
        nc.psum_tensor([128, 256], f32) as ps1,
        nc.semaphore("pe_sem") as pe_sem,
        nc.semaphore("cp_sem") as cp_sem,
        nc.semaphore("out_sem") as out_sem,
    ):
        with ExitStack() as sems_ctx:
            bsems = [
                sems_ctx.enter_context(nc.semaphore(f"bsem{i}"))
                for i in range(nblocks)
            ]

            def issue_stream(q, who):
                for i, bsz in enumerate(sizes):
                    if owner[i] != who:
                        continue
                    if i >= nslots:
                        q.wait_ge(pe_sem, i - nslots + 1)
                    q.dma_start(
                        out=ring[:, i % nslots, :bsz, :],
                        in_=xt[:, starts[i] : starts[i] + bsz, :],
                    ).then_inc(bsems[i], 16)

            with nc.Block() as block:

                @block.sync
                def _(s):
                    issue_stream(s, "sp")
                    s.wait_ge(cp_sem, 2)
                    d = s.dma_start(out=gram[:], in_=outt[:])
                    if end_wait:
                        d.then_inc(out_sem, 16)
                        s.wait_ge(out_sem, 16)

                @block.scalar
                def _(sc):
                    issue_stream(sc, "act")
                    sc.wait_ge(pe_sem, nblocks)
                    nc.scalar.copy(
                        out=outt[:, 128:256], in_=ps1[:, 128:256]
                    ).then_inc(cp_sem, 1)

                @block.tensor
                def _(te):
                    for i, bsz in enumerate(sizes):
                        te.wait_ge(bsems[i], 16)
                        last = None
                        for tl in range(bsz):
                            tcur = starts[i] + tl
                            mov = ring[:, i % nslots, tl, :]
                            for ps, goff in ((ps0, 0), (ps1, 128)):
                                last = nc.tensor.matmul(
                                    ps[:, :],
                                    ring[
                                        :, i % nslots, tl, goff : goff + 128
                                    ],
                                    mov,
                                    start=(tcur == 0),
                                    stop=(tcur == t_per_core - 1),
                                    skip_group_check=True,
                                )
                        last.then_inc(pe_sem, 1)

                @block.vector
                def _(v):
                    v.wait_ge(pe_sem, nblocks)
                    nc.vector.tensor_copy(
                        out=outt[:, 0:128], in_=ps0[:, 0:128]
                    ).then_inc(cp_sem, 1)

    nc.compile()
    _nc_cache[key] = nc
    return nc


def build_nc_v5(t_per_core=T_PER_CORE, ring_chunks=160, end_wait=True):
    """v5 = v4 with a flat chunk-position ring (no per-block slots): block i
    lives at ring chunk starts[i] % ring_chunks, sized so no block wraps.
    Credits are computed from real chunk distances, so with a 160-chunk ring
    every taper DMA is issued far ahead of need and the stream has no
    end-of-kernel serialization."""
    key = ("v5", t_per_core, ring_chunks, end_wait)
    if key in _nc_cache:
        return _nc_cache[key]

    sizes = [2, 2, 4, 8, 16] + [32] * 6 + [16, 8, 4, 2, 1, 1]
    assert sum(sizes) == t_per_core
    nblocks = len(sizes)
    starts = []
    t = 0
    for b in sizes:
        starts.append(t)
        t += b
    for i, b in enumerate(sizes):
        assert starts[i] % ring_chunks + b <= ring_chunks, (i, starts[i], b)
    owner = ["sp" if (i < 2 or i % 2 == 1) else "act" for i in range(nblocks)]

    # credit threshold: block i may load once PE has consumed through chunk
    # starts[i] + size - ring_chunks, i.e. pe_sem >= (number of whole blocks
    # covering those chunks)
    def credit(i):
        need = starts[i] + sizes[i] - ring_chunks
        if need <= 0:
            return 0
        j = 0
        while starts[j] < need:
            j += 1
        return j  # pe_sem counts fully-consumed blocks

    f32 = mybir.dt.float32
    f32r = mybir.dt.float32r

    nc = bacc.Bacc(None, target_bir_lowering=False, debug=False)
    xt = nc.dram_tensor("xt", [128, t_per_core, ROWS], f32r, kind="ExternalInput")
    gram = nc.dram_tensor("gram", [128, 256], f32, kind="ExternalOutput")

    with (
        nc.sbuf_tensor([128, ring_chunks, ROWS], f32r) as ring,
        nc.sbuf_tensor([128, 256], f32) as outt,
        nc.psum_tensor([128, 256], f32) as ps0,
        nc.psum_tensor([128, 256], f32) as ps1,
        nc.semaphore("pe_sem") as pe_sem,
        nc.semaphore("cp_sem") as cp_sem,
        nc.semaphore("out_sem") as out_sem,
    ):
        with ExitStack() as sems_ctx:
            bsems = [
                sems_ctx.enter_context(nc.semaphore(f"bsem{i}"))
                for i in range(nblocks)
            ]

            def issue_stream(q, who):
                for i, bsz in enumerate(sizes):
                    if owner[i] != who:
                        continue
                    c = credit(i)
                    if c > 0:
                        q.wait_ge(pe_sem, c)
                    pos = starts[i] % ring_chunks
                    q.dma_start(
                        out=ring[:, pos : pos + bsz, :],
                        in_=xt[:, starts[i] : starts[i] + bsz, :],
                    ).then_inc(bsems[i], 16)

            with nc.Block() as block:

                @block.sync
                def _(s):
                    issue_stream(s, "sp")
                    s.wait_ge(cp_sem, 2)
                    d = s.dma_start(out=gram[:], in_=outt[:])
                    if end_wait:
                        d.then_inc(out_sem, 16)
                        s.wait_ge(out_sem, 16)

                @block.scalar
                def _(sc):
                    issue_stream(sc, "act")
                    sc.wait_ge(pe_sem, nblocks)
                    nc.scalar.copy(
                        out=outt[:, 128:256], in_=ps1[:, 128:256]
                    ).then_inc(cp_sem, 1)

                @block.tensor
                def _(te):
                    for i, bsz in enumerate(sizes):
                        te.wait_ge(bsems[i], 16)
                        last = None
                        for tl in range(bsz):
                            tcur = starts[i] + tl
                            pos = starts[i] % ring_chunks + tl
                            mov = ring[:, pos, :]
                            for ps, goff in ((ps0, 0), (ps1, 128)):
                                last = nc.tensor.matmul(
                                    ps[:, :],
                                    ring[:, pos, goff : goff + 128],
                                    mov,
                                    start=(tcur == 0),
                                    stop=(tcur == t_per_core - 1),
                                    skip_group_check=True,
                                )
                        last.then_inc(pe_sem, 1)

                @block.vector
                def _(v):
                    v.wait_ge(pe_sem, nblocks)
                    nc.vector.tensor_copy(
                        out=outt[:, 0:128], in_=ps0[:, 0:128]
                    ).then_inc(cp_sem, 1)

    nc.compile()
    _nc_cache[key] = nc
    return nc


def build_nc_hwdge(t_per_core=T_PER_CORE, warmup_mms=128, nstage=3):
    """HWDGE loads (immune to the SWDGE descriptor-ring engine-7/15
    contention): fp32 staged via a 3-slot ring, cast to bf16 on DVE into the
    resident xbuf, PE consumes per block. Same math as build_nc_raw."""
    if t_per_core == T_PER_CORE:
        blocks = [16] * 15 + [8, 4, 4]
    else:
        blocks = [t_per_core // 2] * 2
    assert sum(blocks) == t_per_core
    key = ("hwdge", t_per_core, warmup_mms, nstage)
    if key in _nc_cache:
        return _nc_cache[key]

    nblocks = len(blocks)
    max_b = max(blocks)
    f32 = mybir.dt.float32
    bf16 = mybir.dt.bfloat16

    nc = bacc.Bacc(None, target_bir_lowering=False, debug=False)
    xt = nc.dram_tensor("xt", [128, t_per_core, ROWS], f32, kind="ExternalInput")
    gram = nc.dram_tensor("gram", [128, 256], f32, kind="ExternalOutput")

    starts = []
    t = 0
    for b in blocks:
        starts.append(t)
        t += b

    with (
        nc.sbuf_tensor([128, t_per_core, ROWS], bf16) as xbuf,
        nc.sbuf_tensor([128, nstage, max_b, ROWS], f32) as stage,
        nc.sbuf_tensor([128, 128], bf16) as warm_buf,
        nc.sbuf_tensor([128, 256], f32) as outt,
        nc.psum_tensor([128, 128], f32) as ps0,
        nc.psum_tensor([128, 128], f32) as ps1,
        nc.psum_tensor([128, 128], f32) as ps_warm,
        nc.semaphore("warm_sem") as warm_sem,
        nc.semaphore("cast_done") as cast_done,
        nc.semaphore("mm_sem") as mm_sem,
        nc.semaphore("cp_sem") as cp_sem,
        nc.semaphore("out_sem") as out_sem,
    ):
        with ExitStack() as sems_ctx:
            ssems = [
                sems_ctx.enter_context(nc.semaphore(f"ssem{s}"))
                for s in range(nstage)
            ]

            with nc.Block() as block:

                @block.gpsimd
                def _(g):
                    g.memset(warm_buf[:], 0.0).then_inc(warm_sem, 1)

                @block.sync
                def _(s):
                    for i, bsz in enumerate(blocks):
                        if i >= nstage:
                            # slot free once its previous block is cast
                            s.wait_ge(cast_done, i - nstage + 1)
                        s.dma_start(
                            out=stage[:, i % nstage, :bsz, :],
                            in_=xt[:, starts[i] : starts[i] + bsz, :],
                        ).then_inc(ssems[i % nstage], 16)
                    # output: wait for both PSUM copies, DMA out, drain
                    s.wait_ge(cp_sem, 2)
                    s.dma_start(out=gram[:], in_=outt[:]).then_inc(out_sem, 16)
                    s.wait_ge(out_sem, 16)

                @block.vector
                def _(v):
                    for i, bsz in enumerate(blocks):
                        v.wait_ge(ssems[i % nstage], 16 * (i // nstage + 1))
                        nc.vector.tensor_copy(
                            out=xbuf[:, starts[i] : starts[i] + bsz, :],
                            in_=stage[:, i % nstage, :bsz, :],
                        ).then_inc(cast_done, 1)

                @block.tensor
                def _(te):
                    te.wait_ge(warm_sem, 1)
                    for _ in range(warmup_mms):
                        nc.tensor.matmul(
                            ps_warm[:], warm_buf[:], warm_buf[:], start=True, stop=True
                        )
                    for i, bsz in enumerate(blocks):
                        te.wait_ge(cast_done, i + 1)
                        last = None
                        for tl in range(bsz):
                            tcur = starts[i] + tl
                            for ps, goff in ((ps0, 0), (ps1, 128)):
                                sl = xbuf[:, tcur, goff : goff + 128]
                                last = nc.tensor.matmul(
                                    ps[:],
                                    sl,
                                    sl,
                                    start=(tcur == 0),
                                    stop=(tcur == t_per_core - 1),
                                )
                        if i == nblocks - 1:
                            last.then_inc(mm_sem, 1)

                @block.scalar
                def _(sc):
                    sc.wait_ge(mm_sem, 1)
                    nc.scalar.copy(out=outt[:, 0:128], in_=ps0[:]).then_inc(cp_sem, 1)
                    nc.scalar.copy(out=outt[:, 128:256], in_=ps1[:]).then_inc(
                        cp_sem, 1
                    )

    nc.compile()
    _nc_cache[key] = nc
    return nc


def shard_inputs(pred):
    """[32, 8, 512, 512] fp32 -> per-core [128, T_PER_CORE, 256] arrays.

    Per-core layout: xt[p, t, m] = x[m, c*32768 + t*128 + p] where
    x = pred.reshape(256, 262144). Done in cache-friendly stages.
    """
    x = np.ascontiguousarray(pred, dtype=np.float32).reshape(ROWS, L // 128, 128)
    # stage 1: [m, T, p] -> [T, m, p]   (inner 512B runs are contiguous)
    g = np.ascontiguousarray(x.transpose(1, 0, 2))
    # stage 2: [T, m, p] -> [T, p, m]   (per-T 128 KiB slice, cache resident)
    h = np.ascontiguousarray(g.transpose(0, 2, 1))
    # stage 3: [c*t, p, m] -> [c, p, t, m]  (inner 1 KiB contiguous runs)
    xt = np.ascontiguousarray(
        h.reshape(N_CORES, T_PER_CORE, 128, ROWS).transpose(0, 2, 1, 3)
    )
    return xt


def postprocess(gram_list):
    """Sum per-core partial Grams and reduce to the scalar loss."""
    d = np.zeros((128, 256), dtype=np.float64)
    for garr in gram_list:
        d += np.asarray(garr, dtype=np.float64)
    total = 0.0
    for b in range(B):
        g, j = divmod(b, 16)
        blk = d[8 * j : 8 * j + 8, g * 128 + 8 * j : g * 128 + 8 * j + 8]
        norms = np.sqrt(np.maximum(np.diag(blk), 0.0))
        denom = np.maximum(norms, EPS)
        gn = blk / np.outer(denom, denom)
        np.fill_diagonal(gn, 1.0)
        total += gn.sum()
    return np.asarray(total / (B * NMAP * NMAP), dtype=np.float32)


KERNEL_MODE = os.environ.get("KERNEL_MODE", "v2")


def run(pred, trace=False, **spmd_kwargs):
    pred = np.asarray(pred, dtype=np.float32)
    assert pred.shape == (B, NMAP, H, W), pred.shape
    if KERNEL_MODE == "raw":
        nc = build_nc_raw()
    elif KERNEL_MODE == "hwdge":
        nc = build_nc_hwdge()
    elif KERNEL_MODE == "v2":
        nc = build_nc_v2(
            end_wait=os.environ.get("END_WAIT", "0") == "1",
            big_block=int(os.environ.get("V2_BIG", "32")),
        )
    elif KERNEL_MODE == "v3":
        nc = build_nc_v3(
            end_wait=os.environ.get("END_WAIT", "1") == "1",
            owners=os.environ.get("V3_OWNERS", "bal"),
        )
    elif KERNEL_MODE == "v4":
        nc = build_nc_v4(end_wait=os.environ.get("END_WAIT", "1") == "1")
    elif KERNEL_MODE == "v5":
        nc = build_nc_v5(end_wait=os.environ.get("END_WAIT", "1") == "1")
    else:
        nc = build_nc()
    xt = shard_inputs(pred)
    in_maps = [{"xt": xt[c]} for c in range(N_CORES)]
    res = run_bass_kernel_spmd(
        nc, in_maps, core_ids=list(range(N_CORES)), trace=trace, **spmd_kwargs
    )
    value = postprocess([r["gram"] for r in res.results])
    return value, res


def kernel(pred):
    value, _ = run(pred, trace=False)
    return value

